# revision 8
# baseline (speedup 1.0000x reference)
"""Trainium2 Bass kernel for the EnhancedGNNEncoder (3-layer HydroConv GNN).

Strategy (8 NeuronCores, SPMD):
  - Nodes range-partitioned across cores (dst-sharding). Each core aggregates
    messages for its own nodes, computes the dense update for its slice, and
    an AllGather rebuilds the full node table for the next layer's gathers.
  - The node table is stored bf16 padded to 128 features per row (256 B rows,
    the dma_gather minimum element size), so gathers land directly in bf16
    and the per-edge weight multiply runs in place on the gathered tile.
  - The dst-gather of the reference (w * (h[src] - h[dst])) is eliminated
    algebraically: agg[n] = sum_e w_e h[src_e] - wdeg[n] h[n]. The second
    term is computed on-chip from the resident own-slice (scale by -wdeg,
    transpose on the tensor engine, add into agg) instead of as gathered
    self-edges -- saving ~3% of gather descriptors.
  - Per-edge weights w_e = softplus(edge_attr @ emlp_w + emlp_b) depend only
    on inputs, so they are computed host-side and streamed per-edge.
  - The dense phase (linear + relu), block LayerNorm + residual, the final
    fc, and the slice writeback are all interleaved into the gather phase:
    buckets are ordered so the largest bucket runs last, and each window's
    dense update fires as soon as its final group is flushed. This keeps the
    GpSimd engine (descriptor generation -- the bottleneck) streaming with
    minimal idle at layer boundaries.

The instruction stream is identical on all cores (SPMD); per-core variation
lives in the input tensors. Per-(bucket,window) tile counts are max-reduced
over cores and padded with null edges (w=0).
"""

import math

import numpy as np

D = 64
L = 3
C = 8
WIN = 128
BUCKET = 32768
EPS = 1e-5
CH = 32   # gather-chunk size in 128-edge tiles
BLK = 14  # windows per LayerNorm/writeback block

_CACHE = {}


def _softplus(z):
    return np.logaddexp(0.0, z)


def _prep(x, edge_index, edge_attr, lin_w, lin_b, emlp_w, emlp_b, gamma, beta,
          fc_w, fc_b):
    import ml_dtypes
    BF = ml_dtypes.bfloat16

    N = x.shape[0]
    E = edge_index.shape[1]
    NW = math.ceil(N / (C * WIN))
    SLICE = NW * WIN
    NPAD = C * SLICE
    NB = math.ceil(NPAD / BUCKET)

    src = np.ascontiguousarray(edge_index[0]).astype(np.int64)
    dst = np.ascontiguousarray(edge_index[1]).astype(np.int64)
    ea = np.asarray(edge_attr, dtype=np.float32)

    # per-layer edge weights + per-node weighted degree
    w_layers = np.empty((L, E), dtype=np.float32)
    wdeg = np.empty((L, NPAD), dtype=np.float32)
    for l in range(L):
        z = ea @ np.asarray(emlp_w[l, 0], dtype=np.float32) + float(emlp_b[l, 0])
        w_layers[l] = _softplus(z).astype(np.float32)
        wdeg[l] = np.bincount(dst, weights=w_layers[l].astype(np.float64),
                              minlength=NPAD).astype(np.float32)
    # negated, per-core [128, L, NW] layout (node = c*SLICE + w*128 + p)
    nwdeg = (-wdeg).reshape(L, C, NW, WIN)
    nwdeg = np.transpose(nwdeg, (1, 3, 0, 2)).copy()  # [C, 128, L, NW]

    core_of = dst // SLICE

    per_core = []
    counts = np.zeros((C, NB, NW), dtype=np.int64)
    for c in range(C):
        m = core_of == c
        s_c = src[m]
        d_c = dst[m]
        w_c = w_layers[:, m]
        b_c = s_c // BUCKET
        wl_c = (d_c - c * SLICE) // WIN
        order = np.lexsort((wl_c, b_c))
        s_c, d_c, w_c = s_c[order], d_c[order], w_c[:, order]
        b_c, wl_c = b_c[order], wl_c[order]
        np.add.at(counts[c], (b_c, wl_c), 1)
        per_core.append((s_c, d_c, w_c, b_c, wl_c))

    maxcnt = counts.max(axis=0)  # [NB, NW]
    tiles = np.where(maxcnt > 0, (maxcnt + 127) // 128, 0).astype(np.int64)
    # bucket order: largest bucket LAST so dense-phase interleaving spreads
    bucket_tiles = tiles.sum(axis=1)
    border = sorted(range(NB), key=lambda b: (bucket_tiles[b], b))
    # group schedule shared across cores
    groups = []  # (bucket, window, n_tiles, tile_start)
    tpos = 0
    for b in border:
        for w in range(NW):
            t = int(tiles[b, w])
            if t == 0:
                continue
            groups.append((b, w, t, tpos))
            tpos += t
    TOT_T = tpos
    TOT = TOT_T * 128

    # last group index per window (dense fires after this group's flush)
    lastgroup = {}
    for gi, (b, w, t, ts) in enumerate(groups):
        lastgroup[w] = gi

    # fill per-core streams
    idx16 = np.zeros((C, TOT), dtype=np.int16)
    dstloc = np.full((C, TOT), -1.0, dtype=np.float32)
    wvals = np.zeros((C, L, TOT), dtype=np.float32)
    for c in range(C):
        s_c, d_c, w_c, b_c, wl_c = per_core[c]
        # edges sorted by (b, w) lexicographic; groups are in border order
        starts = {}
        epos = 0
        for b in range(NB):
            for w in range(NW):
                n = int(counts[c, b, w])
                starts[(b, w)] = (epos, n)
                epos += n
        assert epos == len(s_c)
        for (b, w, t, tstart) in groups:
            epos, n = starts[(b, w)]
            if n:
                sl = slice(epos, epos + n)
                o = tstart * 128
                idx16[c, o:o + n] = (s_c[sl] - b * BUCKET).astype(np.int16)
                dstloc[c, o:o + n] = (d_c[sl] - (c * SLICE + w * WIN)).astype(np.float32)
                wvals[c, :, o:o + n] = w_c[:, sl]

    # device layouts
    # wrapped gather indices: edge i -> [i % 16, i // 16], replicated x8
    idx_wrapped = np.zeros((C, 128, TOT // 16), dtype=np.int16)
    for c in range(C):
        w16 = idx16[c].reshape(TOT // 16, 16).T  # [16, TOT//16]
        idx_wrapped[c] = np.tile(w16, (8, 1))
    # per-tile-major: [128, TOT_T]: (p, t) = edge t*128+p
    dstloc_t = np.transpose(dstloc.reshape(C, TOT_T, 128), (0, 2, 1)).astype(BF)
    wvals_t = np.transpose(wvals.reshape(C, L, TOT_T, 128), (0, 1, 3, 2)).astype(BF)

    # chunks: consecutive tile runs within one bucket (in border order)
    chunks = []  # (bucket, tile_start, n_tiles)
    for b in border:
        bt = [g for g in groups if g[0] == b]
        if not bt:
            continue
        b0 = bt[0][3]
        bn = bt[-1][3] + bt[-1][2]
        t = b0
        while t < bn:
            ct = min(CH, bn - t)
            chunks.append((b, t, ct))
            t += ct

    # node table: bf16 padded to 128 features (256B rows)
    x_pad = np.zeros((NPAD, 128), dtype=BF)
    x_pad[:N, :D] = np.asarray(x, dtype=np.float32).astype(BF)
    x_f32 = np.zeros((NPAD, D), dtype=np.float32)
    x_f32[:N] = np.asarray(x, dtype=np.float32)
    x_own = np.transpose(
        x_f32.reshape(C, NW, 128, D), (0, 2, 1, 3)).copy()  # [C, 128, NW, 64]

    iota = np.broadcast_to(np.arange(128, dtype=np.float32), (128, 1, 128)).astype(BF)
    id64 = np.eye(64, dtype=np.float32)
    id128 = np.eye(128, dtype=np.float32)
    lwT = np.transpose(np.asarray(lin_w, dtype=np.float32), (0, 2, 1)).astype(BF).copy()
    fwT = np.asarray(fc_w, dtype=np.float32).T.astype(BF).copy()

    gamma = np.asarray(gamma, dtype=np.float32)
    beta = np.asarray(beta, dtype=np.float32)
    ln_trivial = bool(np.all(gamma == 1.0) and np.all(beta == 0.0))

    meta = dict(N=N, NW=NW, SLICE=SLICE, NPAD=NPAD, NB=NB, TOT_T=TOT_T,
                groups=tuple(groups), chunks=tuple(chunks),
                lastgroup=tuple(sorted(lastgroup.items())),
                ln_trivial=ln_trivial)

    in_maps = []
    for c in range(C):
        in_maps.append({
            "x_pad": x_pad,
            "x_own": x_own[c],
            "idx_w": idx_wrapped[c],
            "dstloc": dstloc_t[c],
            "wv": wvals_t[c],
            "nwdeg": nwdeg[c],
            "iota": iota,
            "id64": id64,
            "id128": id128,
            "lwT": lwT,
            "lb": np.asarray(lin_b, dtype=np.float32),
            "fwT": fwT,
            "fb": np.asarray(fc_b, dtype=np.float32).reshape(64, 1),
            "gm": np.broadcast_to(gamma[:, None, :], (L, 128, D)).copy(),
            "bt": np.broadcast_to(beta[:, None, :], (L, 128, D)).copy(),
        })
    return meta, in_maps


def _split_multi_waits(nc, mybir):
    """This walrus build rejects >1 sync-wait per instruction; hoist extras
    onto single-wait NOPs inserted just before, same engine."""
    ctr = 0
    for bbw in nc.bb_map.values():
        bb = bbw.bb
        insts = bb.instructions
        new = []
        changed = False
        for inst in insts:
            si = inst.sync_info
            waits = list(si.on_wait) if si and si.on_wait else []
            if len(waits) > 1:
                changed = True
                for w in waits[:-1]:
                    ctr += 1
                    new.append(mybir.InstNoOp(
                        name=f"I-waitsplit-{ctr}",
                        engine=inst.engine,
                        sync_info=mybir.SyncInfo(on_wait=[w], on_update=[]),
                    ))
                si.on_wait = [waits[-1]]
            new.append(inst)
        if changed:
            bb.instructions = new
    return nc


def _build(meta, split_waits=True, n_layers=L):
    import concourse.bass as bass
    import concourse.mybir as mybir
    from concourse import library_config
    from concourse.library_overlay import lower_extended_insts
    from concourse.tile import TileContext

    NW = meta["NW"]
    SLICE = meta["SLICE"]
    NPAD = meta["NPAD"]
    NB = meta["NB"]
    TOT_T = meta["TOT_T"]
    groups = meta["groups"]
    chunks = meta["chunks"]
    lastgroup = dict(meta["lastgroup"])
    ln_trivial = meta["ln_trivial"]
    TOT = TOT_T * 128
    NBLK = math.ceil(NW / BLK)

    F32 = mybir.dt.float32
    BF = mybir.dt.bfloat16
    I16 = mybir.dt.int16
    AF = mybir.ActivationFunctionType
    OP = mybir.AluOpType

    nc = bass.Bass(num_devices=C, num_swdge_queues=4)

    x_pad = nc.declare_dram_parameter("x_pad", [NPAD, 128], BF, isOutput=False)
    x_own = nc.declare_dram_parameter("x_own", [128, NW, D], F32, isOutput=False)
    idx_w = nc.declare_dram_parameter("idx_w", [128, TOT // 16], I16, isOutput=False)
    dstloc = nc.declare_dram_parameter("dstloc", [128, TOT_T], BF, isOutput=False)
    wv = nc.declare_dram_parameter("wv", [L, 128, TOT_T], BF, isOutput=False)
    nwdeg = nc.declare_dram_parameter("nwdeg", [128, L, NW], F32, isOutput=False)
    iota = nc.declare_dram_parameter("iota", [128, 1, 128], BF, isOutput=False)
    id64 = nc.declare_dram_parameter("id64", [64, 64], F32, isOutput=False)
    id128 = nc.declare_dram_parameter("id128", [128, 128], F32, isOutput=False)
    lwT = nc.declare_dram_parameter("lwT", [L, 64, 64], BF, isOutput=False)
    lb = nc.declare_dram_parameter("lb", [L, 64], F32, isOutput=False)
    fwT = nc.declare_dram_parameter("fwT", [64, 64], BF, isOutput=False)
    fb = nc.declare_dram_parameter("fb", [64, 1], F32, isOutput=False)
    if not ln_trivial:
        gm = nc.declare_dram_parameter("gm", [L, 128, 64], F32, isOutput=False)
        bt = nc.declare_dram_parameter("bt", [L, 128, 64], F32, isOutput=False)
    out = nc.declare_dram_parameter("out", [128, NW, D], F32, isOutput=True)

    if n_layers > 1:
        tabs = [
            nc.dram_tensor("tabA", [NPAD, 128], BF, addr_space="Shared"),
            nc.dram_tensor("tabB", [NPAD, 128], BF, addr_space="Shared"),
        ]
        slice_outs = [
            nc.dram_tensor("slice0", [SLICE, 128], BF),
            nc.dram_tensor("slice1", [SLICE, 128], BF),
        ]
    else:
        tabs, slice_outs = [x_pad, x_pad], []

    nc.gpsimd.load_library(library_config.mlp)

    with TileContext(nc) as tc:
        with (
            tc.tile_pool(name="const", bufs=1) as cpool,
            tc.tile_pool(name="big", bufs=1) as bigp,
            tc.tile_pool(name="gat", bufs=4) as gpool,
            tc.tile_pool(name="oh", bufs=4) as opool,
            tc.tile_pool(name="strm", bufs=6) as stp,
            tc.tile_pool(name="dense", bufs=3) as dpool,
            tc.tile_pool(name="lnp", bufs=2) as lnp,
            tc.tile_pool(name="stgp", bufs=2) as stgp,
            tc.tile_pool(name="psagg", bufs=4, space="PSUM") as ps_agg,
            tc.tile_pool(name="psd", bufs=2, space="PSUM") as ps_d,
            tc.tile_pool(name="pst", bufs=2, space="PSUM") as ps_t,
        ):
            # constants
            iota_t = cpool.tile([128, 1, 128], BF)
            nc.sync.dma_start(out=iota_t[:], in_=iota[:, :, :])
            id64_t = cpool.tile([64, 64], F32)
            nc.sync.dma_start(out=id64_t[:], in_=id64[:, :])
            id128_t = cpool.tile([128, 128], F32)
            nc.sync.dma_start(out=id128_t[:], in_=id128[:, :])
            nwdeg_t = cpool.tile([128, L, NW], F32)
            nc.sync.dma_start(out=nwdeg_t[:], in_=nwdeg[:, :, :])
            lwT_ts = []
            for l in range(L):
                t = cpool.tile([64, 64], BF, tag=f"lwT{l}")
                nc.sync.dma_start(out=t[:], in_=lwT[l, :, :])
                lwT_ts.append(t)
            lb_ts = []
            for l in range(L):
                t = cpool.tile([64, 1], F32, tag=f"lb{l}")
                nc.sync.dma_start(out=t[:], in_=lb[l, :, None])
                lb_ts.append(t)
            fwT_t = cpool.tile([64, 64], BF)
            nc.sync.dma_start(out=fwT_t[:], in_=fwT[:, :])
            fb_t = cpool.tile([64, 1], F32)
            nc.sync.dma_start(out=fb_t[:], in_=fb[:, :])
            gm_ts, bt_ts = [], []
            if not ln_trivial:
                for l in range(L):
                    g_ = cpool.tile([128, 64], F32, tag=f"gm{l}")
                    nc.sync.dma_start(out=g_[:], in_=gm[l, :, :])
                    gm_ts.append(g_)
                    b_ = cpool.tile([128, 64], F32, tag=f"bt{l}")
                    nc.sync.dma_start(out=b_[:], in_=bt[l, :, :])
                    bt_ts.append(b_)

            eps_t = cpool.tile([128, 1], F32)
            nc.vector.memset(eps_t[:], EPS)
            # one register per distinct gather size, reused across all calls
            nidx_regs = {}
            for (_b, _t0, _ct) in chunks:
                v = _ct * 128
                if v not in nidx_regs:
                    nidx_regs[v] = nc.gpsimd.to_reg(v)

            own = [bigp.tile([128, NW, D], F32, tag="own_a", name="own_a"),
                   bigp.tile([128, NW, D], F32, tag="own_b", name="own_b")]
            nc.sync.dma_start(out=own[0][:], in_=x_own[:, :, :])
            agg = bigp.tile([64, NW * 128], BF, tag="agg", name="agg")
            stage = bigp.tile([128, NW, D], F32, tag="stage", name="stage")

            for l in range(n_layers):
                tab_in = x_pad if l == 0 else tabs[l - 1]
                own_cur = own[l % 2]
                own_nxt = own[(l + 1) % 2]
                last = l == n_layers - 1

                nc.vector.memset(agg[:], 0.0)

                # self term: agg[:, w] += transpose(own_cur[:, w, :] * -wdeg)
                for w in range(NW):
                    sc = dpool.tile([128, 64], F32, tag="sc", name="sc")
                    nc.vector.tensor_tensor(
                        out=sc[:], in0=own_cur[:, w, :],
                        in1=nwdeg_t[:, l, w, None].to_broadcast([128, 64]),
                        op=OP.mult)
                    pt = ps_t.tile([64, 128], F32, tag="pst", name="pst")
                    nc.tensor.transpose(pt[:], sc[:], id128_t[:])
                    nc.vector.tensor_tensor(
                        out=agg[:, w * 128:(w + 1) * 128],
                        in0=agg[:, w * 128:(w + 1) * 128],
                        in1=pt[:], op=OP.add)

                win_done = set()
                blocks_done = [0]

                def dense_window(w):
                    pd = ps_d.tile([64, 128], F32, tag="psd", name="psd")
                    nc.tensor.matmul(pd[:], lhsT=lwT_ts[l][:],
                                     rhs=agg[:, w * 128:(w + 1) * 128],
                                     start=True, stop=True)
                    rT = dpool.tile([64, 128], F32, tag="rT", name="rT")
                    nc.scalar.activation(rT[:], pd[:], AF.Relu,
                                         bias=lb_ts[l][:, 0:1])
                    pt = ps_t.tile([128, 64], F32, tag="pst", name="pst")
                    nc.tensor.transpose(pt[:], rT[:], id64_t[:])
                    nc.scalar.copy(own_nxt[:, w, :], pt[:])

                def finish_block(k):
                    w0 = k * BLK
                    w1 = min(w0 + BLK, NW)
                    nb = w1 - w0
                    blk = own_nxt[:, w0:w1, :]
                    mu_s = dpool.tile([128, BLK], F32, tag="mu", name="mu")
                    nc.vector.tensor_reduce(mu_s[:, :nb], blk,
                                            axis=mybir.AxisListType.X, op=OP.add)
                    sq = lnp.tile([128, BLK, D], BF, tag="sq", name="sq")
                    nc.scalar.activation(sq[:, :nb, :], blk, AF.Square)
                    ssq = dpool.tile([128, BLK], F32, tag="ssq", name="ssq")
                    nc.vector.tensor_reduce(ssq[:, :nb], sq[:, :nb, :],
                                            axis=mybir.AxisListType.X, op=OP.add)
                    a2 = dpool.tile([128, BLK], F32, tag="a2", name="a2")
                    nc.vector.tensor_tensor(out=a2[:, :nb], in0=mu_s[:, :nb],
                                            in1=mu_s[:, :nb], op=OP.mult)
                    bvar = dpool.tile([128, BLK], F32, tag="bvar", name="bvar")
                    nc.vector.scalar_tensor_tensor(
                        out=bvar[:, :nb], in0=a2[:, :nb], scalar=-1.0 / D,
                        in1=ssq[:, :nb], op0=OP.mult, op1=OP.add)
                    std = dpool.tile([128, BLK], F32, tag="std", name="std")
                    nc.scalar.activation(std[:, :nb], bvar[:, :nb], AF.Sqrt,
                                         bias=eps_t[:, 0:1], scale=1.0 / D)
                    rstd = dpool.tile([128, BLK], F32, tag="rstd", name="rstd")
                    nc.vector.reciprocal(rstd[:, :nb], std[:, :nb])
                    xc = lnp.tile([128, BLK, D], BF, tag="xc", name="xc")
                    nc.vector.scalar_tensor_tensor(
                        out=xc[:, :nb, :],
                        in0=mu_s[:, :nb, None].to_broadcast([128, nb, D]),
                        scalar=-1.0 / D, in1=blk,
                        op0=OP.mult, op1=OP.add)
                    nc.vector.tensor_tensor(
                        out=blk, in0=xc[:, :nb, :],
                        in1=rstd[:, :nb, None].to_broadcast([128, nb, D]),
                        op=OP.mult)
                    if not ln_trivial:
                        nc.vector.tensor_tensor(
                            out=blk, in0=blk,
                            in1=gm_ts[l][:, None, :].to_broadcast([128, nb, D]),
                            op=OP.mult)
                        nc.vector.tensor_tensor(
                            out=blk, in0=blk,
                            in1=bt_ts[l][:, None, :].to_broadcast([128, nb, D]),
                            op=OP.add)
                    nc.vector.tensor_tensor(out=blk, in0=blk,
                                            in1=own_cur[:, w0:w1, :], op=OP.add)
                    if not last:
                        # write padded bf16 slice rows for the next table
                        stg = stgp.tile([128, BLK, 128], BF, tag="stg", name="stg")
                        nc.scalar.copy(stg[:, :nb, 0:64], blk)
                        so_ap = slice_outs[l].ap().rearrange(
                            "(w p) f -> p w f", p=128)
                        nc.sync.dma_start(out=so_ap[:, w0:w1, :],
                                          in_=stg[:, :nb, :])
                    else:
                        # final fc on this block, then store to out
                        for w in range(w0, w1):
                            ptf = ps_t.tile([64, 128], F32, tag="pst", name="pst")
                            nc.tensor.transpose(ptf[:], own_nxt[:, w, :],
                                                id128_t[:])
                            hT = dpool.tile([64, 128], BF, tag="hT", name="hT")
                            nc.scalar.copy(hT[:], ptf[:])
                            po = ps_d.tile([64, 128], F32, tag="psd", name="psd")
                            nc.tensor.matmul(po[:], lhsT=fwT_t[:], rhs=hT[:],
                                             start=True, stop=True)
                            ob = dpool.tile([64, 128], F32, tag="ob", name="ob")
                            nc.vector.tensor_scalar_add(ob[:], po[:],
                                                        fb_t[:, 0:1])
                            pq = ps_t.tile([128, 64], F32, tag="pst", name="pst")
                            nc.tensor.transpose(pq[:], ob[:], id64_t[:])
                            nc.scalar.copy(stage[:, w, :], pq[:])
                        nc.sync.dma_start(out=out[:, w0:w1, :],
                                          in_=stage[:, w0:w1, :])

                open_ps = {}
                for ci_, (b, t0, ct) in enumerate(chunks):
                    nidx = ct * 128
                    idx_t = stp.tile([128, CH * 8], I16, tag="idx", name="idx")
                    nc.sync.dma_start(out=idx_t[:, :ct * 8],
                                      in_=idx_w[:, t0 * 8:(t0 + ct) * 8])
                    dst_t = stp.tile([128, CH], BF, tag="dst", name="dst")
                    nc.scalar.dma_start(out=dst_t[:, :ct],
                                        in_=dstloc[:, t0:t0 + ct])
                    w_t = stp.tile([128, CH], BF, tag="w", name="w")
                    nc.scalar.dma_start(out=w_t[:, :ct],
                                        in_=wv[l, :, t0:t0 + ct])

                    gat = gpool.tile([128, CH, 128], BF, tag="gat", name="gat")
                    brows = min(BUCKET, NPAD - b * BUCKET)
                    nc.gpsimd.dma_gather(
                        out_ap=gat[:, :ct, :],
                        in_ap=tab_in[b * BUCKET:b * BUCKET + brows, :],
                        idxs_ap=idx_t[:, :ct * 8],
                        num_idxs=nidx,
                        num_idxs_reg=nidx_regs[nidx],
                        elem_size=128,
                        single_packet=False,
                        queue_num=ci_ % 4,
                    )
                    # scale messages in place (features 0:64 of each row)
                    nc.vector.tensor_tensor(
                        out=gat[:, :ct, 0:64],
                        in0=gat[:, :ct, 0:64],
                        in1=w_t[:, :ct, None].to_broadcast([128, ct, 64]),
                        op=OP.mult,
                    )
                    oh = opool.tile([128, CH, 128], BF, tag="oh", name="oh")
                    nc.vector.tensor_tensor(
                        out=oh[:, :ct, :],
                        in0=dst_t[:, :ct, None].to_broadcast([128, ct, 128]),
                        in1=iota_t[:].to_broadcast([128, ct, 128]),
                        op=OP.is_equal,
                    )
                    # matmuls per tile
                    for gi, (gb, gw, gt, gstart) in enumerate(groups):
                        if gstart + gt <= t0 or gstart >= t0 + ct:
                            continue
                        lo = max(gstart, t0)
                        hi = min(gstart + gt, t0 + ct)
                        if gstart >= t0:
                            open_ps[gi] = ps_agg.tile([64, 128], F32,
                                                      tag="psagg", name="psagg")
                        ps = open_ps[gi]
                        for t in range(lo, hi):
                            ti = t - t0
                            nc.tensor.matmul(
                                ps[:],
                                lhsT=gat[:, ti, 0:64],
                                rhs=oh[:, ti, :],
                                start=(t == gstart),
                                stop=(t == gstart + gt - 1),
                            )
                        if gstart + gt <= t0 + ct:
                            # group complete: flush into agg
                            nc.vector.tensor_tensor(
                                out=agg[:, gw * 128:(gw + 1) * 128],
                                in0=agg[:, gw * 128:(gw + 1) * 128],
                                in1=ps[:],
                                op=OP.add,
                            )
                            del open_ps[gi]
                            # window complete -> dense; block complete -> LN
                            if lastgroup.get(gw) == gi:
                                dense_window(gw)
                                win_done.add(gw)
                                while (blocks_done[0] < NBLK and all(
                                        w_ in win_done for w_ in
                                        range(blocks_done[0] * BLK,
                                              min((blocks_done[0] + 1) * BLK,
                                                  NW)))):
                                    finish_block(blocks_done[0])
                                    blocks_done[0] += 1
                assert not open_ps

                # windows with no groups at all (shouldn't happen, but safe)
                for w in range(NW):
                    if w not in win_done:
                        dense_window(w)
                        win_done.add(w)
                while blocks_done[0] < NBLK:
                    finish_block(blocks_done[0])
                    blocks_done[0] += 1

                if not last:
                    nc.gpsimd.collective_compute(
                        "AllGather",
                        mybir.AluOpType.bypass,
                        replica_groups=[list(range(C))],
                        ins=[slice_outs[l][:].opt()],
                        outs=[tabs[l][:].opt()],
                    )

    if split_waits:
        _split_multi_waits(nc, mybir)
    lower_extended_insts(nc)
    return nc


def kernel(**inputs):
    from concourse.bass_utils import run_bass_kernel_spmd

    x = np.asarray(inputs["x"])
    meta, in_maps = _prep(
        x, np.asarray(inputs["edge_index"]), np.asarray(inputs["edge_attr"]),
        np.asarray(inputs["lin_w"]), np.asarray(inputs["lin_b"]),
        np.asarray(inputs["emlp_w"]), np.asarray(inputs["emlp_b"]),
        np.asarray(inputs["gamma"]), np.asarray(inputs["beta"]),
        np.asarray(inputs["fc_w"]), np.asarray(inputs["fc_b"]))

    key = (meta["NW"], meta["TOT_T"], meta["groups"], meta["chunks"],
           meta["ln_trivial"])
    if key not in _CACHE:
        _CACHE[key] = _build(meta)
    nc = _CACHE[key]

    res = run_bass_kernel_spmd(nc, in_maps, list(range(C)))
    N = meta["N"]
    NW = meta["NW"]
    parts = []
    for c in range(C):
        o = np.asarray(res.results[c]["out"])  # [128, NW, 64]
        parts.append(np.transpose(o, (1, 0, 2)).reshape(NW * 128, D))
    full = np.concatenate(parts, axis=0)[:N]
    return full.astype(np.float32)


# revision 31
# speedup vs baseline: 1.0756x; 1.0756x over previous
"""Trainium2 Bass kernel for the EnhancedGNNEncoder (3-layer HydroConv GNN).

Strategy (8 NeuronCores, SPMD):
  - Nodes range-partitioned across cores (dst-sharding). Each core aggregates
    messages for its own nodes, computes the dense update for its slice, and
    an AllGather rebuilds the full node table for the next layer's gathers.
  - The node table is stored bf16 padded to 128 features per row (256 B rows,
    the dma_gather minimum element size), so gathers land directly in bf16
    and the per-edge weight multiply runs in place on the gathered tile.
  - The dst-gather of the reference (w * (h[src] - h[dst])) is eliminated
    algebraically: agg[n] = sum_e w_e h[src_e] - wdeg[n] h[n]. The second
    term is computed on-chip from the resident own-slice (scale by -wdeg,
    transpose on the tensor engine, add into agg) instead of as gathered
    self-edges -- saving ~3% of gather descriptors.
  - Per-edge weights w_e = softplus(edge_attr @ emlp_w + emlp_b) depend only
    on inputs, so they are computed host-side and streamed per-edge.
  - The dense phase (linear + relu), block LayerNorm + residual, the final
    fc, and the slice writeback are all interleaved into the gather phase:
    buckets are ordered so the largest bucket runs last, and each window's
    dense update fires as soon as its final group is flushed. This keeps the
    GpSimd engine (descriptor generation -- the bottleneck) streaming with
    minimal idle at layer boundaries.

The instruction stream is identical on all cores (SPMD); per-core variation
lives in the input tensors. Per-(bucket,window) tile counts are max-reduced
over cores and padded with null edges (w=0).
"""

import math

import numpy as np

D = 64
L = 3
C = 8
WIN = 128
BUCKET = 32768
EPS = 1e-5
CH = 32       # gather-chunk size in 128-edge tiles
BLK = 14      # windows per LayerNorm/writeback block
SPLIT_W = 64  # windows in the first (early) AllGather = buckets 0-1

_CACHE = {}


def _softplus(z):
    return np.logaddexp(0.0, z)


def _prep(x, edge_index, edge_attr, lin_w, lin_b, emlp_w, emlp_b, gamma, beta,
          fc_w, fc_b):
    import ml_dtypes
    BF = ml_dtypes.bfloat16

    N = x.shape[0]
    E = edge_index.shape[1]
    NW = math.ceil(N / (C * WIN))
    SLICE = NW * WIN
    NPAD = C * SLICE
    NB = math.ceil(NPAD / BUCKET)

    src = np.ascontiguousarray(edge_index[0]).astype(np.int64)
    dst = np.ascontiguousarray(edge_index[1]).astype(np.int64)
    ea = np.asarray(edge_attr, dtype=np.float32)

    # per-layer edge weights + per-node weighted degree
    w_layers = np.empty((L, E), dtype=np.float32)
    wdeg = np.empty((L, NPAD), dtype=np.float32)
    for l in range(L):
        z = ea @ np.asarray(emlp_w[l, 0], dtype=np.float32) + float(emlp_b[l, 0])
        w_layers[l] = _softplus(z).astype(np.float32)
        wdeg[l] = np.bincount(dst, weights=w_layers[l].astype(np.float64),
                              minlength=NPAD).astype(np.float32)
    # negated, per-core [128, L, NW] layout (node = c*SLICE + w*128 + p)
    nwdeg = (-wdeg).reshape(L, C, NW, WIN)
    nwdeg = np.transpose(nwdeg, (1, 3, 0, 2)).copy()  # [C, 128, L, NW]

    core_of = dst // SLICE

    # split global renumbering: the node table is the concatenation of a "lo"
    # half (each core's windows [0, SPLIT_W), rank-major) and a "hi" half
    # (windows [SPLIT_W, NW), rank-major). Each half is produced by ONE
    # contiguous AllGather; lo = gather buckets 0..1 fires early (mid-layer),
    # so the next layer's first buckets depend only on it.
    n_all = np.arange(NPAD, dtype=np.int64)
    c_all = n_all // SLICE
    loc = n_all % SLICE
    w_all = loc // WIN
    p_all = loc % WIN
    LO_ROWS = SPLIT_W * C * WIN
    gid_of = np.where(
        w_all < SPLIT_W,
        c_all * (SPLIT_W * WIN) + w_all * WIN + p_all,
        LO_ROWS + c_all * ((NW - SPLIT_W) * WIN) + (w_all - SPLIT_W) * WIN + p_all)
    src_g = gid_of[src]

    per_core = []
    counts = np.zeros((C, NB, NW), dtype=np.int64)
    for c in range(C):
        m = core_of == c
        s_c = src_g[m]
        d_c = dst[m]
        w_c = w_layers[:, m]
        b_c = s_c // BUCKET
        wl_c = (d_c - c * SLICE) // WIN
        order = np.lexsort((wl_c, b_c))
        s_c, d_c, w_c = s_c[order], d_c[order], w_c[:, order]
        b_c, wl_c = b_c[order], wl_c[order]
        np.add.at(counts[c], (b_c, wl_c), 1)
        per_core.append((s_c, d_c, w_c, b_c, wl_c))

    maxcnt = counts.max(axis=0)  # [NB, NW]
    tiles = np.where(maxcnt > 0, (maxcnt + 127) // 128, 0).astype(np.int64)
    # bucket order: early-collective buckets (0,1) first, largest bucket last
    bucket_tiles = tiles.sum(axis=1)
    early = [b for b in range(NB) if (b + 1) * BUCKET <= SPLIT_W * C * WIN]
    rest = sorted((b for b in range(NB) if b not in early),
                  key=lambda b: (bucket_tiles[b], b))
    border = early + rest
    # group schedule shared across cores
    groups = []  # (bucket, window, n_tiles, tile_start)
    tpos = 0
    for b in border:
        for w in range(NW):
            t = int(tiles[b, w])
            if t == 0:
                continue
            groups.append((b, w, t, tpos))
            tpos += t
    TOT_T = tpos
    TOT = TOT_T * 128

    # last group index per window (dense fires after this group's flush);
    # first group index per window (self-term write issued when it opens)
    lastgroup = {}
    firstgroup = {}
    for gi, (b, w, t, ts) in enumerate(groups):
        lastgroup[w] = gi
        if w not in firstgroup:
            firstgroup[w] = gi

    # fill per-core streams
    idx16 = np.zeros((C, TOT), dtype=np.int16)
    dstloc = np.full((C, TOT), -1.0, dtype=np.float32)
    wvals = np.zeros((C, L, TOT), dtype=np.float32)
    for c in range(C):
        s_c, d_c, w_c, b_c, wl_c = per_core[c]
        # edges sorted by (b, w) lexicographic; groups are in border order
        starts = {}
        epos = 0
        for b in range(NB):
            for w in range(NW):
                n = int(counts[c, b, w])
                starts[(b, w)] = (epos, n)
                epos += n
        assert epos == len(s_c)
        for (b, w, t, tstart) in groups:
            epos, n = starts[(b, w)]
            if n:
                sl = slice(epos, epos + n)
                o = tstart * 128
                idx16[c, o:o + n] = (s_c[sl] - b * BUCKET).astype(np.int16)
                dstloc[c, o:o + n] = (d_c[sl] - (c * SLICE + w * WIN)).astype(np.float32)
                wvals[c, :, o:o + n] = w_c[:, sl]

    # chunks: consecutive tile runs within one bucket (in border order)
    chunks = []  # (bucket, tile_start, n_tiles)
    for b in border:
        bt = [g for g in groups if g[0] == b]
        if not bt:
            continue
        b0 = bt[0][3]
        bn = bt[-1][3] + bt[-1][2]
        t = b0
        while t < bn:
            ct = min(CH, bn - t)
            chunks.append((b, t, ct))
            t += ct

    # mark chunk-trailing pad slots with idx -1: the Q7 desc-gen kernel trims
    # trailing negatives per call, skipping their descriptors on cores with
    # fewer real edges (interior pads stay 0 -- gathered then zeroed by w=0)
    pad = dstloc < 0  # [C, TOT]  (trim disabled for bisect)

    # device layouts
    # wrapped gather indices: edge i -> [i % 16, i // 16], replicated x8
    idx_wrapped = np.zeros((C, 128, TOT // 16), dtype=np.int16)
    for c in range(C):
        w16 = idx16[c].reshape(TOT // 16, 16).T  # [16, TOT//16]
        idx_wrapped[c] = np.tile(w16, (8, 1))
    # per-tile-major: [128, TOT_T]: (p, t) = edge t*128+p
    dstloc_t = np.transpose(dstloc.reshape(C, TOT_T, 128), (0, 2, 1)).astype(BF)
    wvals_t = np.transpose(wvals.reshape(C, L, TOT_T, 128), (0, 1, 3, 2)).astype(BF)

    # node table: bf16 padded to 128 features (256B rows), rows in gid order
    x_pad = np.zeros((NPAD, 128), dtype=BF)
    x_pad[gid_of[:N], :D] = np.asarray(x, dtype=np.float32).astype(BF)
    x_f32 = np.zeros((NPAD, D), dtype=np.float32)
    x_f32[:N] = np.asarray(x, dtype=np.float32)
    x_own = np.transpose(
        x_f32.reshape(C, NW, 128, D), (0, 2, 1, 3)).copy()  # [C, 128, NW, 64]

    iota = np.broadcast_to(np.arange(128, dtype=np.float32), (128, 1, 128)).astype(BF)
    id64 = np.eye(64, dtype=np.float32)
    id128 = np.eye(128, dtype=np.float32)
    lwT = np.transpose(np.asarray(lin_w, dtype=np.float32), (0, 2, 1)).astype(BF).copy()
    fwT = np.asarray(fc_w, dtype=np.float32).T.astype(BF).copy()

    gamma = np.asarray(gamma, dtype=np.float32)
    beta = np.asarray(beta, dtype=np.float32)
    ln_trivial = bool(np.all(gamma == 1.0) and np.all(beta == 0.0))

    meta = dict(N=N, NW=NW, SLICE=SLICE, NPAD=NPAD, NB=NB, TOT_T=TOT_T,
                groups=tuple(groups), chunks=tuple(chunks),
                lastgroup=tuple(sorted(lastgroup.items())),
                firstgroup=tuple(sorted(firstgroup.items())),
                ln_trivial=ln_trivial)

    in_maps = []
    for c in range(C):
        in_maps.append({
            "x_pad": x_pad,
            "x_own": x_own[c],
            "idx_w": idx_wrapped[c],
            "dstloc": dstloc_t[c],
            "wv": wvals_t[c],
            "nwdeg": nwdeg[c],
            "iota": iota,
            "id64": id64,
            "id128": id128,
            "lwT": lwT,
            "lb": np.asarray(lin_b, dtype=np.float32),
            "fwT": fwT,
            "fb": np.asarray(fc_b, dtype=np.float32).reshape(64, 1),
            "gm": np.broadcast_to(gamma[:, None, :], (L, 128, D)).copy(),
            "bt": np.broadcast_to(beta[:, None, :], (L, 128, D)).copy(),
        })
    return meta, in_maps


def _split_multi_waits(nc, mybir):
    """This walrus build rejects >1 sync-wait per instruction; hoist extras
    onto single-wait NOPs inserted just before, same engine."""
    ctr = 0
    for bbw in nc.bb_map.values():
        bb = bbw.bb
        insts = bb.instructions
        new = []
        changed = False
        for inst in insts:
            si = inst.sync_info
            waits = list(si.on_wait) if si and si.on_wait else []
            if len(waits) > 1:
                changed = True
                for w in waits[:-1]:
                    ctr += 1
                    new.append(mybir.InstNoOp(
                        name=f"I-waitsplit-{ctr}",
                        engine=inst.engine,
                        sync_info=mybir.SyncInfo(on_wait=[w], on_update=[]),
                    ))
                si.on_wait = [waits[-1]]
            new.append(inst)
        if changed:
            bb.instructions = new
    return nc


def _build(meta, split_waits=True, n_layers=L):
    import concourse.bass as bass
    import concourse.mybir as mybir
    from concourse import library_config
    from concourse.library_overlay import lower_extended_insts
    from concourse.tile import TileContext

    NW = meta["NW"]
    SLICE = meta["SLICE"]
    NPAD = meta["NPAD"]
    NB = meta["NB"]
    TOT_T = meta["TOT_T"]
    groups = meta["groups"]
    chunks = meta["chunks"]
    lastgroup = dict(meta["lastgroup"])
    firstgroup = dict(meta["firstgroup"])
    ln_trivial = meta["ln_trivial"]
    TOT = TOT_T * 128
    NBLK = math.ceil(NW / BLK)
    # block index after which the early AllGather (windows < SPLIT_W) fires
    ABLK = math.ceil(SPLIT_W / BLK) - 1

    F32 = mybir.dt.float32
    BF = mybir.dt.bfloat16
    I16 = mybir.dt.int16
    AF = mybir.ActivationFunctionType
    OP = mybir.AluOpType

    nc = bass.Bass(num_devices=C, num_swdge_queues=4)

    x_pad = nc.declare_dram_parameter("x_pad", [NPAD, 128], BF, isOutput=False)
    x_own = nc.declare_dram_parameter("x_own", [128, NW, D], F32, isOutput=False)
    idx_w = nc.declare_dram_parameter("idx_w", [128, TOT // 16], I16, isOutput=False)
    dstloc = nc.declare_dram_parameter("dstloc", [128, TOT_T], BF, isOutput=False)
    wv = nc.declare_dram_parameter("wv", [L, 128, TOT_T], BF, isOutput=False)
    nwdeg = nc.declare_dram_parameter("nwdeg", [128, L, NW], F32, isOutput=False)
    iota = nc.declare_dram_parameter("iota", [128, 1, 128], BF, isOutput=False)
    id64 = nc.declare_dram_parameter("id64", [64, 64], F32, isOutput=False)
    id128 = nc.declare_dram_parameter("id128", [128, 128], F32, isOutput=False)
    lwT = nc.declare_dram_parameter("lwT", [L, 64, 64], BF, isOutput=False)
    lb = nc.declare_dram_parameter("lb", [L, 64], F32, isOutput=False)
    fwT = nc.declare_dram_parameter("fwT", [64, 64], BF, isOutput=False)
    fb = nc.declare_dram_parameter("fb", [64, 1], F32, isOutput=False)
    if not ln_trivial:
        gm = nc.declare_dram_parameter("gm", [L, 128, 64], F32, isOutput=False)
        bt = nc.declare_dram_parameter("bt", [L, 128, 64], F32, isOutput=False)
    out = nc.declare_dram_parameter("out", [128, NW, D], F32, isOutput=True)

    LO_ROWS = SPLIT_W * C * WIN
    HI_ROWS = NPAD - LO_ROWS
    if n_layers > 1:
        tabs_lo = [
            nc.dram_tensor("tabA_lo", [LO_ROWS, 128], BF, addr_space="Shared"),
            nc.dram_tensor("tabB_lo", [LO_ROWS, 128], BF, addr_space="Shared"),
        ]
        tabs_hi = [
            nc.dram_tensor("tabA_hi", [HI_ROWS, 128], BF, addr_space="Shared"),
            nc.dram_tensor("tabB_hi", [HI_ROWS, 128], BF, addr_space="Shared"),
        ]
        slice_outs = [
            nc.dram_tensor("slice0", [SLICE, 128], BF),
            nc.dram_tensor("slice1", [SLICE, 128], BF),
        ]
    else:
        tabs_lo, tabs_hi, slice_outs = [x_pad, x_pad], [x_pad, x_pad], []

    nc.gpsimd.load_library(library_config.mlp)

    with TileContext(nc) as tc:
        with (
            tc.tile_pool(name="const", bufs=1) as cpool,
            tc.tile_pool(name="big", bufs=1) as bigp,
            tc.tile_pool(name="gat", bufs=4) as gpool,
            tc.tile_pool(name="msg", bufs=4) as mpool,
            tc.tile_pool(name="oh", bufs=3) as opool,
            tc.tile_pool(name="strm", bufs=6) as stp,
            tc.tile_pool(name="dense", bufs=3) as dpool,
            tc.tile_pool(name="lnp", bufs=1) as lnp,
            tc.tile_pool(name="stgp", bufs=2) as stgp,
            tc.tile_pool(name="psagg", bufs=4, space="PSUM") as ps_agg,
            tc.tile_pool(name="psd", bufs=2, space="PSUM") as ps_d,
            tc.tile_pool(name="pst", bufs=2, space="PSUM") as ps_t,
        ):
            # constants
            iota_t = cpool.tile([128, 1, 128], BF)
            nc.sync.dma_start(out=iota_t[:], in_=iota[:, :, :])
            id64_t = cpool.tile([64, 64], F32)
            nc.sync.dma_start(out=id64_t[:], in_=id64[:, :])
            id128_t = cpool.tile([128, 128], F32)
            nc.sync.dma_start(out=id128_t[:], in_=id128[:, :])
            nwdeg_t = cpool.tile([128, L, NW], F32)
            nc.sync.dma_start(out=nwdeg_t[:], in_=nwdeg[:, :, :])
            lwT_ts = []
            for l in range(L):
                t = cpool.tile([64, 64], BF, tag=f"lwT{l}")
                nc.sync.dma_start(out=t[:], in_=lwT[l, :, :])
                lwT_ts.append(t)
            lb_ts = []
            for l in range(L):
                t = cpool.tile([64, 1], F32, tag=f"lb{l}")
                nc.sync.dma_start(out=t[:], in_=lb[l, :, None])
                lb_ts.append(t)
            fwT_t = cpool.tile([64, 64], BF)
            nc.sync.dma_start(out=fwT_t[:], in_=fwT[:, :])
            fb_t = cpool.tile([64, 1], F32)
            nc.sync.dma_start(out=fb_t[:], in_=fb[:, :])
            gm_ts, bt_ts = [], []
            if not ln_trivial:
                for l in range(L):
                    g_ = cpool.tile([128, 64], F32, tag=f"gm{l}")
                    nc.sync.dma_start(out=g_[:], in_=gm[l, :, :])
                    gm_ts.append(g_)
                    b_ = cpool.tile([128, 64], F32, tag=f"bt{l}")
                    nc.sync.dma_start(out=b_[:], in_=bt[l, :, :])
                    bt_ts.append(b_)

            eps_t = cpool.tile([128, 1], F32)
            nc.vector.memset(eps_t[:], EPS)
            # one register per distinct gather size, reused across all calls
            nidx_regs = {}
            for (_b, _t0, _ct) in chunks:
                v = _ct * 128
                if v not in nidx_regs:
                    nidx_regs[v] = nc.gpsimd.to_reg(v)

            own = [bigp.tile([128, NW, D], F32, tag="own_a", name="own_a"),
                   bigp.tile([128, NW, D], F32, tag="own_b", name="own_b")]
            nc.sync.dma_start(out=own[0][:], in_=x_own[:, :, :])
            agg = bigp.tile([64, NW * 128], BF, tag="agg", name="agg")
            stage = bigp.tile([128, NW, D], F32, tag="stage", name="stage")

            for l in range(n_layers):
                own_cur = own[l % 2]
                own_nxt = own[(l + 1) % 2]
                last = l == n_layers - 1

                win_done = set()
                self_done = set()
                blocks_done = [0]

                def self_write(w):
                    # agg[:, w] = transpose(own_cur[:, w, :] * -wdeg)
                    # (first write of the window; flushes add on top)
                    sc = dpool.tile([128, 64], F32, tag="sc", name="sc")
                    nc.vector.tensor_tensor(
                        out=sc[:], in0=own_cur[:, w, :],
                        in1=nwdeg_t[:, l, w, None].to_broadcast([128, 64]),
                        op=OP.mult)
                    pt = ps_t.tile([64, 128], F32, tag="pst", name="pst")
                    nc.tensor.transpose(pt[:], sc[:], id128_t[:])
                    nc.scalar.copy(agg[:, w * 128:(w + 1) * 128], pt[:])
                    self_done.add(w)

                def dense_window(w):
                    pd = ps_d.tile([64, 128], F32, tag="psd", name="psd")
                    nc.tensor.matmul(pd[:], lhsT=lwT_ts[l][:],
                                     rhs=agg[:, w * 128:(w + 1) * 128],
                                     start=True, stop=True)
                    rT = dpool.tile([64, 128], F32, tag="rT", name="rT")
                    nc.scalar.activation(rT[:], pd[:], AF.Relu,
                                         bias=lb_ts[l][:, 0:1])
                    pt = ps_t.tile([128, 64], F32, tag="pst", name="pst")
                    nc.tensor.transpose(pt[:], rT[:], id64_t[:])
                    nc.scalar.copy(own_nxt[:, w, :], pt[:])

                def finish_block(k):
                    w0 = k * BLK
                    w1 = min(w0 + BLK, NW)
                    nb = w1 - w0
                    blk = own_nxt[:, w0:w1, :]
                    mu_s = dpool.tile([128, BLK], F32, tag="mu", name="mu")
                    nc.vector.tensor_reduce(mu_s[:, :nb], blk,
                                            axis=mybir.AxisListType.X, op=OP.add)
                    sq = lnp.tile([128, BLK, D], BF, tag="sq", name="sq")
                    nc.scalar.activation(sq[:, :nb, :], blk, AF.Square)
                    ssq = dpool.tile([128, BLK], F32, tag="ssq", name="ssq")
                    nc.vector.tensor_reduce(ssq[:, :nb], sq[:, :nb, :],
                                            axis=mybir.AxisListType.X, op=OP.add)
                    a2 = dpool.tile([128, BLK], F32, tag="a2", name="a2")
                    nc.vector.tensor_tensor(out=a2[:, :nb], in0=mu_s[:, :nb],
                                            in1=mu_s[:, :nb], op=OP.mult)
                    bvar = dpool.tile([128, BLK], F32, tag="bvar", name="bvar")
                    nc.vector.scalar_tensor_tensor(
                        out=bvar[:, :nb], in0=a2[:, :nb], scalar=-1.0 / D,
                        in1=ssq[:, :nb], op0=OP.mult, op1=OP.add)
                    std = dpool.tile([128, BLK], F32, tag="std", name="std")
                    nc.scalar.activation(std[:, :nb], bvar[:, :nb], AF.Sqrt,
                                         bias=eps_t[:, 0:1], scale=1.0 / D)
                    rstd = dpool.tile([128, BLK], F32, tag="rstd", name="rstd")
                    nc.vector.reciprocal(rstd[:, :nb], std[:, :nb])
                    xc = lnp.tile([128, BLK, D], BF, tag="xc", name="xc")
                    nc.vector.scalar_tensor_tensor(
                        out=xc[:, :nb, :],
                        in0=mu_s[:, :nb, None].to_broadcast([128, nb, D]),
                        scalar=-1.0 / D, in1=blk,
                        op0=OP.mult, op1=OP.add)
                    nc.vector.tensor_tensor(
                        out=blk, in0=xc[:, :nb, :],
                        in1=rstd[:, :nb, None].to_broadcast([128, nb, D]),
                        op=OP.mult)
                    if not ln_trivial:
                        nc.vector.tensor_tensor(
                            out=blk, in0=blk,
                            in1=gm_ts[l][:, None, :].to_broadcast([128, nb, D]),
                            op=OP.mult)
                        nc.vector.tensor_tensor(
                            out=blk, in0=blk,
                            in1=bt_ts[l][:, None, :].to_broadcast([128, nb, D]),
                            op=OP.add)
                    nc.vector.tensor_tensor(out=blk, in0=blk,
                                            in1=own_cur[:, w0:w1, :], op=OP.add)
                    if not last:
                        # write padded bf16 slice rows for the next table
                        stg = stgp.tile([128, BLK, 128], BF, tag="stg", name="stg")
                        nc.scalar.copy(stg[:, :nb, 0:64], blk)
                        so_ap = slice_outs[l].ap().rearrange(
                            "(w p) f -> p w f", p=128)
                        nc.sync.dma_start(out=so_ap[:, w0:w1, :],
                                          in_=stg[:, :nb, :])
                        # early AllGather: windows [0, SPLIT_W) feed buckets
                        # 0..1 of the next layer; fires mid-gather-phase and
                        # overlaps the remaining gathers of this layer
                        if k == ABLK:
                            nc.gpsimd.collective_compute(
                                "AllGather",
                                mybir.AluOpType.bypass,
                                replica_groups=[list(range(C))],
                                ins=[slice_outs[l][0:SPLIT_W * 128, :].opt()],
                                outs=[tabs_lo[l][:].opt()],
                            )
                    else:
                        # final fc on this block, then store to out
                        for w in range(w0, w1):
                            ptf = ps_t.tile([64, 128], F32, tag="pst", name="pst")
                            nc.tensor.transpose(ptf[:], own_nxt[:, w, :],
                                                id128_t[:])
                            hT = dpool.tile([64, 128], BF, tag="hT", name="hT")
                            nc.scalar.copy(hT[:], ptf[:])
                            po = ps_d.tile([64, 128], F32, tag="psd", name="psd")
                            nc.tensor.matmul(po[:], lhsT=fwT_t[:], rhs=hT[:],
                                             start=True, stop=True)
                            ob = dpool.tile([64, 128], F32, tag="ob", name="ob")
                            nc.vector.tensor_scalar_add(ob[:], po[:],
                                                        fb_t[:, 0:1])
                            pq = ps_t.tile([128, 64], F32, tag="pst", name="pst")
                            nc.tensor.transpose(pq[:], ob[:], id64_t[:])
                            nc.scalar.copy(stage[:, w, :], pq[:])
                        nc.sync.dma_start(out=out[:, w0:w1, :],
                                          in_=stage[:, w0:w1, :])

                open_ps = {}
                for ci_, (b, t0, ct) in enumerate(chunks):
                    nidx = ct * 128
                    idx_t = stp.tile([128, CH * 8], I16, tag="idx", name="idx")
                    nc.sync.dma_start(out=idx_t[:, :ct * 8],
                                      in_=idx_w[:, t0 * 8:(t0 + ct) * 8])
                    dst_t = stp.tile([128, CH], BF, tag="dst", name="dst")
                    nc.scalar.dma_start(out=dst_t[:, :ct],
                                        in_=dstloc[:, t0:t0 + ct])
                    w_t = stp.tile([128, CH], BF, tag="w", name="w")
                    nc.scalar.dma_start(out=w_t[:, :ct],
                                        in_=wv[l, :, t0:t0 + ct])

                    gat = gpool.tile([128, CH, 128], BF, tag="gat", name="gat")
                    brows = min(BUCKET, NPAD - b * BUCKET)
                    if l == 0:
                        tab_ap = x_pad[b * BUCKET:b * BUCKET + brows, :]
                    elif b * BUCKET < LO_ROWS:
                        tab_ap = tabs_lo[l - 1][b * BUCKET:b * BUCKET + brows, :]
                    else:
                        r0 = b * BUCKET - LO_ROWS
                        tab_ap = tabs_hi[l - 1][r0:r0 + brows, :]
                    nc.gpsimd.dma_gather(
                        out_ap=gat[:, :ct, :],
                        in_ap=tab_ap,
                        idxs_ap=idx_t[:, :ct * 8],
                        num_idxs=nidx,
                        num_idxs_reg=nidx_regs[nidx],
                        elem_size=128,
                        single_packet=False,
                        queue_num=ci_ % 4,
                    )
                    # scale messages (features 0:64 of each gathered row)
                    msgs = mpool.tile([128, CH, 64], BF, tag="msgs", name="msgs")
                    nc.vector.tensor_tensor(
                        out=msgs[:, :ct, :],
                        in0=gat[:, :ct, 0:64],
                        in1=w_t[:, :ct, None].to_broadcast([128, ct, 64]),
                        op=OP.mult,
                    )
                    oh = opool.tile([128, CH, 128], BF, tag="oh", name="oh")
                    nc.vector.tensor_tensor(
                        out=oh[:, :ct, :],
                        in0=dst_t[:, :ct, None].to_broadcast([128, ct, 128]),
                        in1=iota_t[:].to_broadcast([128, ct, 128]),
                        op=OP.is_equal,
                    )
                    # matmuls per tile
                    for gi, (gb, gw, gt, gstart) in enumerate(groups):
                        if gstart + gt <= t0 or gstart >= t0 + ct:
                            continue
                        lo = max(gstart, t0)
                        hi = min(gstart + gt, t0 + ct)
                        if gstart >= t0:
                            open_ps[gi] = ps_agg.tile([64, 128], F32,
                                                      tag="psagg", name="psagg")
                            if firstgroup.get(gw) == gi:
                                self_write(gw)
                        ps = open_ps[gi]
                        for t in range(lo, hi):
                            ti = t - t0
                            nc.tensor.matmul(
                                ps[:],
                                lhsT=msgs[:, ti, :],
                                rhs=oh[:, ti, :],
                                start=(t == gstart),
                                stop=(t == gstart + gt - 1),
                            )
                        if gstart + gt <= t0 + ct:
                            # group complete: flush into agg
                            nc.vector.tensor_tensor(
                                out=agg[:, gw * 128:(gw + 1) * 128],
                                in0=agg[:, gw * 128:(gw + 1) * 128],
                                in1=ps[:],
                                op=OP.add,
                            )
                            del open_ps[gi]
                            # window complete -> dense; block complete -> LN
                            if lastgroup.get(gw) == gi:
                                dense_window(gw)
                                win_done.add(gw)
                                while (blocks_done[0] < NBLK and all(
                                        w_ in win_done for w_ in
                                        range(blocks_done[0] * BLK,
                                              min((blocks_done[0] + 1) * BLK,
                                                  NW)))):
                                    finish_block(blocks_done[0])
                                    blocks_done[0] += 1
                assert not open_ps

                # windows with no groups at all (shouldn't happen, but safe)
                for w in range(NW):
                    if w not in win_done:
                        if w not in self_done:
                            self_write(w)
                        dense_window(w)
                        win_done.add(w)
                while blocks_done[0] < NBLK:
                    finish_block(blocks_done[0])
                    blocks_done[0] += 1

                if not last:
                    nc.gpsimd.collective_compute(
                        "AllGather",
                        mybir.AluOpType.bypass,
                        replica_groups=[list(range(C))],
                        ins=[slice_outs[l][SPLIT_W * 128:SLICE, :].opt()],
                        outs=[tabs_hi[l][:].opt()],
                    )

    if split_waits:
        _split_multi_waits(nc, mybir)
    lower_extended_insts(nc)
    return nc


def kernel(**inputs):
    from concourse.bass_utils import run_bass_kernel_spmd

    x = np.asarray(inputs["x"])
    meta, in_maps = _prep(
        x, np.asarray(inputs["edge_index"]), np.asarray(inputs["edge_attr"]),
        np.asarray(inputs["lin_w"]), np.asarray(inputs["lin_b"]),
        np.asarray(inputs["emlp_w"]), np.asarray(inputs["emlp_b"]),
        np.asarray(inputs["gamma"]), np.asarray(inputs["beta"]),
        np.asarray(inputs["fc_w"]), np.asarray(inputs["fc_b"]))

    key = (meta["NW"], meta["TOT_T"], meta["groups"], meta["chunks"],
           meta["ln_trivial"])
    if key not in _CACHE:
        _CACHE[key] = _build(meta)
    nc = _CACHE[key]

    res = run_bass_kernel_spmd(nc, in_maps, list(range(C)))
    N = meta["N"]
    NW = meta["NW"]
    parts = []
    for c in range(C):
        o = np.asarray(res.results[c]["out"])  # [128, NW, 64]
        parts.append(np.transpose(o, (1, 0, 2)).reshape(NW * 128, D))
    full = np.concatenate(parts, axis=0)[:N]
    return full.astype(np.float32)


# revision 33
# speedup vs baseline: 1.1175x; 1.0390x over previous
"""Trainium2 Bass kernel for the EnhancedGNNEncoder (3-layer HydroConv GNN).

Strategy (8 NeuronCores, SPMD):
  - Nodes range-partitioned across cores (dst-sharding). Each core aggregates
    messages for its own nodes, computes the dense update for its slice, and
    an AllGather rebuilds the full node table for the next layer's gathers.
  - The node table is stored bf16 padded to 128 features per row (256 B rows,
    the dma_gather minimum element size), so gathers land directly in bf16
    and the per-edge weight multiply runs in place on the gathered tile.
  - The dst-gather of the reference (w * (h[src] - h[dst])) is eliminated
    algebraically: agg[n] = sum_e w_e h[src_e] - wdeg[n] h[n]. The second
    term is computed on-chip from the resident own-slice (scale by -wdeg,
    transpose on the tensor engine, add into agg) instead of as gathered
    self-edges -- saving ~3% of gather descriptors.
  - Per-edge weights w_e = softplus(edge_attr @ emlp_w + emlp_b) depend only
    on inputs, so they are computed host-side and streamed per-edge.
  - The dense phase (linear + relu), block LayerNorm + residual, the final
    fc, and the slice writeback are all interleaved into the gather phase:
    buckets are ordered so the largest bucket runs last, and each window's
    dense update fires as soon as its final group is flushed. This keeps the
    GpSimd engine (descriptor generation -- the bottleneck) streaming with
    minimal idle at layer boundaries.

The instruction stream is identical on all cores (SPMD); per-core variation
lives in the input tensors. Per-(bucket,window) tile counts are max-reduced
over cores and padded with null edges (w=0).
"""

import math

import numpy as np

D = 64
L = 3
C = 8
WIN = 128
BUCKET = 28672
EPS = 1e-5
CH = 32       # gather-chunk size in 128-edge tiles
BLK = 14      # windows per LayerNorm/writeback block
BLK_LAST = 7  # smaller blocks in the last layer shrink the kernel tail
SPLIT_W = 56  # windows in the first (early) AllGather = buckets 0-1

_CACHE = {}


def _softplus(z):
    return np.logaddexp(0.0, z)


def _prep(x, edge_index, edge_attr, lin_w, lin_b, emlp_w, emlp_b, gamma, beta,
          fc_w, fc_b):
    import ml_dtypes
    BF = ml_dtypes.bfloat16

    N = x.shape[0]
    E = edge_index.shape[1]
    NW = math.ceil(N / (C * WIN))
    SLICE = NW * WIN
    NPAD = C * SLICE
    NB = math.ceil(NPAD / BUCKET)

    src = np.ascontiguousarray(edge_index[0]).astype(np.int64)
    dst = np.ascontiguousarray(edge_index[1]).astype(np.int64)
    ea = np.asarray(edge_attr, dtype=np.float32)

    # per-layer edge weights + per-node weighted degree
    w_layers = np.empty((L, E), dtype=np.float32)
    wdeg = np.empty((L, NPAD), dtype=np.float32)
    for l in range(L):
        z = ea @ np.asarray(emlp_w[l, 0], dtype=np.float32) + float(emlp_b[l, 0])
        w_layers[l] = _softplus(z).astype(np.float32)
        wdeg[l] = np.bincount(dst, weights=w_layers[l].astype(np.float64),
                              minlength=NPAD).astype(np.float32)
    # negated, per-core [128, L, NW] layout (node = c*SLICE + w*128 + p)
    nwdeg = (-wdeg).reshape(L, C, NW, WIN)
    nwdeg = np.transpose(nwdeg, (1, 3, 0, 2)).copy()  # [C, 128, L, NW]

    core_of = dst // SLICE

    # split global renumbering: the node table is the concatenation of a "lo"
    # half (each core's windows [0, SPLIT_W), rank-major) and a "hi" half
    # (windows [SPLIT_W, NW), rank-major). Each half is produced by ONE
    # contiguous AllGather; lo = gather buckets 0..1 fires early (mid-layer),
    # so the next layer's first buckets depend only on it.
    n_all = np.arange(NPAD, dtype=np.int64)
    c_all = n_all // SLICE
    loc = n_all % SLICE
    w_all = loc // WIN
    p_all = loc % WIN
    LO_ROWS = SPLIT_W * C * WIN
    gid_of = np.where(
        w_all < SPLIT_W,
        c_all * (SPLIT_W * WIN) + w_all * WIN + p_all,
        LO_ROWS + c_all * ((NW - SPLIT_W) * WIN) + (w_all - SPLIT_W) * WIN + p_all)
    src_g = gid_of[src]

    per_core = []
    counts = np.zeros((C, NB, NW), dtype=np.int64)
    for c in range(C):
        m = core_of == c
        s_c = src_g[m]
        d_c = dst[m]
        w_c = w_layers[:, m]
        b_c = s_c // BUCKET
        wl_c = (d_c - c * SLICE) // WIN
        order = np.lexsort((wl_c, b_c))
        s_c, d_c, w_c = s_c[order], d_c[order], w_c[:, order]
        b_c, wl_c = b_c[order], wl_c[order]
        np.add.at(counts[c], (b_c, wl_c), 1)
        per_core.append((s_c, d_c, w_c, b_c, wl_c))

    maxcnt = counts.max(axis=0)  # [NB, NW]
    tiles = np.where(maxcnt > 0, (maxcnt + 127) // 128, 0).astype(np.int64)
    # bucket order: early-collective buckets (0,1) first, largest bucket last
    bucket_tiles = tiles.sum(axis=1)
    early = [b for b in range(NB) if (b + 1) * BUCKET <= SPLIT_W * C * WIN]
    rest = sorted((b for b in range(NB) if b not in early),
                  key=lambda b: (bucket_tiles[b], b))
    border = early + rest
    # group schedule shared across cores
    groups = []  # (bucket, window, n_tiles, tile_start)
    tpos = 0
    for b in border:
        for w in range(NW):
            t = int(tiles[b, w])
            if t == 0:
                continue
            groups.append((b, w, t, tpos))
            tpos += t
    TOT_T = tpos
    TOT = TOT_T * 128

    # last group index per window (dense fires after this group's flush);
    # first group index per window (self-term write issued when it opens)
    lastgroup = {}
    firstgroup = {}
    for gi, (b, w, t, ts) in enumerate(groups):
        lastgroup[w] = gi
        if w not in firstgroup:
            firstgroup[w] = gi

    # fill per-core streams
    idx16 = np.zeros((C, TOT), dtype=np.int16)
    dstloc = np.full((C, TOT), -1.0, dtype=np.float32)
    wvals = np.zeros((C, L, TOT), dtype=np.float32)
    for c in range(C):
        s_c, d_c, w_c, b_c, wl_c = per_core[c]
        # edges sorted by (b, w) lexicographic; groups are in border order
        starts = {}
        epos = 0
        for b in range(NB):
            for w in range(NW):
                n = int(counts[c, b, w])
                starts[(b, w)] = (epos, n)
                epos += n
        assert epos == len(s_c)
        for (b, w, t, tstart) in groups:
            epos, n = starts[(b, w)]
            if n:
                sl = slice(epos, epos + n)
                o = tstart * 128
                idx16[c, o:o + n] = (s_c[sl] - b * BUCKET).astype(np.int16)
                dstloc[c, o:o + n] = (d_c[sl] - (c * SLICE + w * WIN)).astype(np.float32)
                wvals[c, :, o:o + n] = w_c[:, sl]

    # chunks: consecutive tile runs within one bucket (in border order)
    chunks = []  # (bucket, tile_start, n_tiles)
    for b in border:
        bt = [g for g in groups if g[0] == b]
        if not bt:
            continue
        b0 = bt[0][3]
        bn = bt[-1][3] + bt[-1][2]
        t = b0
        while t < bn:
            ct = min(CH, bn - t)
            chunks.append((b, t, ct))
            t += ct

    # mark chunk-trailing pad slots with idx -1: the Q7 desc-gen kernel trims
    # trailing negatives per call, skipping their descriptors on cores with
    # fewer real edges (interior pads stay 0 -- gathered then zeroed by w=0)
    pad = dstloc < 0  # [C, TOT]  (trim disabled for bisect)

    # device layouts
    # wrapped gather indices: edge i -> [i % 16, i // 16], replicated x8
    idx_wrapped = np.zeros((C, 128, TOT // 16), dtype=np.int16)
    for c in range(C):
        w16 = idx16[c].reshape(TOT // 16, 16).T  # [16, TOT//16]
        idx_wrapped[c] = np.tile(w16, (8, 1))
    # per-tile-major: [128, TOT_T]: (p, t) = edge t*128+p
    dstloc_t = np.transpose(dstloc.reshape(C, TOT_T, 128), (0, 2, 1)).astype(BF)
    wvals_t = np.transpose(wvals.reshape(C, L, TOT_T, 128), (0, 1, 3, 2)).astype(BF)

    # node table: bf16 padded to 128 features (256B rows), rows in gid order
    x_pad = np.zeros((NPAD, 128), dtype=BF)
    x_pad[gid_of[:N], :D] = np.asarray(x, dtype=np.float32).astype(BF)
    x_f32 = np.zeros((NPAD, D), dtype=np.float32)
    x_f32[:N] = np.asarray(x, dtype=np.float32)
    x_own = np.transpose(
        x_f32.reshape(C, NW, 128, D), (0, 2, 1, 3)).copy()  # [C, 128, NW, 64]

    iota = np.broadcast_to(np.arange(128, dtype=np.float32), (128, 1, 128)).astype(BF)
    id64 = np.eye(64, dtype=np.float32)
    id128 = np.eye(128, dtype=np.float32)
    lwT = np.transpose(np.asarray(lin_w, dtype=np.float32), (0, 2, 1)).astype(BF).copy()
    fwT = np.asarray(fc_w, dtype=np.float32).T.astype(BF).copy()

    gamma = np.asarray(gamma, dtype=np.float32)
    beta = np.asarray(beta, dtype=np.float32)
    ln_trivial = bool(np.all(gamma == 1.0) and np.all(beta == 0.0))

    meta = dict(N=N, NW=NW, SLICE=SLICE, NPAD=NPAD, NB=NB, TOT_T=TOT_T,
                groups=tuple(groups), chunks=tuple(chunks),
                lastgroup=tuple(sorted(lastgroup.items())),
                firstgroup=tuple(sorted(firstgroup.items())),
                ln_trivial=ln_trivial)

    in_maps = []
    for c in range(C):
        in_maps.append({
            "x_pad": x_pad,
            "x_own": x_own[c],
            "idx_w": idx_wrapped[c],
            "dstloc": dstloc_t[c],
            "wv": wvals_t[c],
            "nwdeg": nwdeg[c],
            "iota": iota,
            "id64": id64,
            "id128": id128,
            "lwT": lwT,
            "lb": np.asarray(lin_b, dtype=np.float32),
            "fwT": fwT,
            "fb": np.asarray(fc_b, dtype=np.float32).reshape(64, 1),
            "gm": np.broadcast_to(gamma[:, None, :], (L, 128, D)).copy(),
            "bt": np.broadcast_to(beta[:, None, :], (L, 128, D)).copy(),
        })
    return meta, in_maps


def _split_multi_waits(nc, mybir):
    """This walrus build rejects >1 sync-wait per instruction; hoist extras
    onto single-wait NOPs inserted just before, same engine."""
    ctr = 0
    for bbw in nc.bb_map.values():
        bb = bbw.bb
        insts = bb.instructions
        new = []
        changed = False
        for inst in insts:
            si = inst.sync_info
            waits = list(si.on_wait) if si and si.on_wait else []
            if len(waits) > 1:
                changed = True
                for w in waits[:-1]:
                    ctr += 1
                    new.append(mybir.InstNoOp(
                        name=f"I-waitsplit-{ctr}",
                        engine=inst.engine,
                        sync_info=mybir.SyncInfo(on_wait=[w], on_update=[]),
                    ))
                si.on_wait = [waits[-1]]
            new.append(inst)
        if changed:
            bb.instructions = new
    return nc


def _build(meta, split_waits=True, n_layers=L):
    import concourse.bass as bass
    import concourse.mybir as mybir
    from concourse import library_config
    from concourse.library_overlay import lower_extended_insts
    from concourse.tile import TileContext

    NW = meta["NW"]
    SLICE = meta["SLICE"]
    NPAD = meta["NPAD"]
    NB = meta["NB"]
    TOT_T = meta["TOT_T"]
    groups = meta["groups"]
    chunks = meta["chunks"]
    lastgroup = dict(meta["lastgroup"])
    firstgroup = dict(meta["firstgroup"])
    ln_trivial = meta["ln_trivial"]
    TOT = TOT_T * 128
    # block index after which the early AllGather (windows < SPLIT_W) fires
    ABLK = math.ceil(SPLIT_W / BLK) - 1

    F32 = mybir.dt.float32
    BF = mybir.dt.bfloat16
    I16 = mybir.dt.int16
    AF = mybir.ActivationFunctionType
    OP = mybir.AluOpType

    nc = bass.Bass(num_devices=C, num_swdge_queues=4)

    x_pad = nc.declare_dram_parameter("x_pad", [NPAD, 128], BF, isOutput=False)
    x_own = nc.declare_dram_parameter("x_own", [128, NW, D], F32, isOutput=False)
    idx_w = nc.declare_dram_parameter("idx_w", [128, TOT // 16], I16, isOutput=False)
    dstloc = nc.declare_dram_parameter("dstloc", [128, TOT_T], BF, isOutput=False)
    wv = nc.declare_dram_parameter("wv", [L, 128, TOT_T], BF, isOutput=False)
    nwdeg = nc.declare_dram_parameter("nwdeg", [128, L, NW], F32, isOutput=False)
    iota = nc.declare_dram_parameter("iota", [128, 1, 128], BF, isOutput=False)
    id64 = nc.declare_dram_parameter("id64", [64, 64], F32, isOutput=False)
    id128 = nc.declare_dram_parameter("id128", [128, 128], F32, isOutput=False)
    lwT = nc.declare_dram_parameter("lwT", [L, 64, 64], BF, isOutput=False)
    lb = nc.declare_dram_parameter("lb", [L, 64], F32, isOutput=False)
    fwT = nc.declare_dram_parameter("fwT", [64, 64], BF, isOutput=False)
    fb = nc.declare_dram_parameter("fb", [64, 1], F32, isOutput=False)
    if not ln_trivial:
        gm = nc.declare_dram_parameter("gm", [L, 128, 64], F32, isOutput=False)
        bt = nc.declare_dram_parameter("bt", [L, 128, 64], F32, isOutput=False)
    out = nc.declare_dram_parameter("out", [128, NW, D], F32, isOutput=True)

    LO_ROWS = SPLIT_W * C * WIN
    HI_ROWS = NPAD - LO_ROWS
    if n_layers > 1:
        tabs_lo = [
            nc.dram_tensor("tabA_lo", [LO_ROWS, 128], BF, addr_space="Shared"),
            nc.dram_tensor("tabB_lo", [LO_ROWS, 128], BF, addr_space="Shared"),
        ]
        tabs_hi = [
            nc.dram_tensor("tabA_hi", [HI_ROWS, 128], BF, addr_space="Shared"),
            nc.dram_tensor("tabB_hi", [HI_ROWS, 128], BF, addr_space="Shared"),
        ]
        slice_outs = [
            nc.dram_tensor("slice0", [SLICE, 128], BF),
            nc.dram_tensor("slice1", [SLICE, 128], BF),
        ]
    else:
        tabs_lo, tabs_hi, slice_outs = [x_pad, x_pad], [x_pad, x_pad], []

    nc.gpsimd.load_library(library_config.mlp)

    with TileContext(nc) as tc:
        with (
            tc.tile_pool(name="const", bufs=1) as cpool,
            tc.tile_pool(name="big", bufs=1) as bigp,
            tc.tile_pool(name="gat", bufs=5) as gpool,
            tc.tile_pool(name="msg", bufs=4) as mpool,
            tc.tile_pool(name="oh", bufs=3) as opool,
            tc.tile_pool(name="strm", bufs=6) as stp,
            tc.tile_pool(name="dense", bufs=3) as dpool,
            tc.tile_pool(name="lnp", bufs=1) as lnp,
            tc.tile_pool(name="stgp", bufs=2) as stgp,
            tc.tile_pool(name="psagg", bufs=4, space="PSUM") as ps_agg,
            tc.tile_pool(name="psd", bufs=2, space="PSUM") as ps_d,
            tc.tile_pool(name="pst", bufs=2, space="PSUM") as ps_t,
        ):
            # constants
            iota_t = cpool.tile([128, 1, 128], BF)
            nc.sync.dma_start(out=iota_t[:], in_=iota[:, :, :])
            id64_t = cpool.tile([64, 64], F32)
            nc.sync.dma_start(out=id64_t[:], in_=id64[:, :])
            id128_t = cpool.tile([128, 128], F32)
            nc.sync.dma_start(out=id128_t[:], in_=id128[:, :])
            nwdeg_t = cpool.tile([128, L, NW], F32)
            nc.sync.dma_start(out=nwdeg_t[:], in_=nwdeg[:, :, :])
            lwT_ts = []
            for l in range(L):
                t = cpool.tile([64, 64], BF, tag=f"lwT{l}")
                nc.sync.dma_start(out=t[:], in_=lwT[l, :, :])
                lwT_ts.append(t)
            lb_ts = []
            for l in range(L):
                t = cpool.tile([64, 1], F32, tag=f"lb{l}")
                nc.sync.dma_start(out=t[:], in_=lb[l, :, None])
                lb_ts.append(t)
            fwT_t = cpool.tile([64, 64], BF)
            nc.sync.dma_start(out=fwT_t[:], in_=fwT[:, :])
            fb_t = cpool.tile([64, 1], F32)
            nc.sync.dma_start(out=fb_t[:], in_=fb[:, :])
            gm_ts, bt_ts = [], []
            if not ln_trivial:
                for l in range(L):
                    g_ = cpool.tile([128, 64], F32, tag=f"gm{l}")
                    nc.sync.dma_start(out=g_[:], in_=gm[l, :, :])
                    gm_ts.append(g_)
                    b_ = cpool.tile([128, 64], F32, tag=f"bt{l}")
                    nc.sync.dma_start(out=b_[:], in_=bt[l, :, :])
                    bt_ts.append(b_)

            eps_t = cpool.tile([128, 1], F32)
            nc.vector.memset(eps_t[:], EPS)
            # one register per distinct gather size, reused across all calls
            nidx_regs = {}
            for (_b, _t0, _ct) in chunks:
                v = _ct * 128
                if v not in nidx_regs:
                    nidx_regs[v] = nc.gpsimd.to_reg(v)

            own = [bigp.tile([128, NW, D], F32, tag="own_a", name="own_a"),
                   bigp.tile([128, NW, D], F32, tag="own_b", name="own_b")]
            nc.sync.dma_start(out=own[0][:], in_=x_own[:, :, :])
            agg = bigp.tile([64, NW * 128], BF, tag="agg", name="agg")
            stage = bigp.tile([128, NW, D], F32, tag="stage", name="stage")

            for l in range(n_layers):
                own_cur = own[l % 2]
                own_nxt = own[(l + 1) % 2]
                last = l == n_layers - 1

                BLKL = BLK_LAST if l == n_layers - 1 else BLK
                NBLK = math.ceil(NW / BLKL)
                win_done = set()
                self_done = set()
                blocks_done = [0]

                def self_write(w):
                    # agg[:, w] = transpose(own_cur[:, w, :] * -wdeg)
                    # (first write of the window; flushes add on top)
                    sc = dpool.tile([128, 64], F32, tag="sc", name="sc")
                    nc.vector.tensor_tensor(
                        out=sc[:], in0=own_cur[:, w, :],
                        in1=nwdeg_t[:, l, w, None].to_broadcast([128, 64]),
                        op=OP.mult)
                    pt = ps_t.tile([64, 128], F32, tag="pst", name="pst")
                    nc.tensor.transpose(pt[:], sc[:], id128_t[:])
                    nc.scalar.copy(agg[:, w * 128:(w + 1) * 128], pt[:])
                    self_done.add(w)

                def dense_window(w):
                    pd = ps_d.tile([64, 128], F32, tag="psd", name="psd")
                    nc.tensor.matmul(pd[:], lhsT=lwT_ts[l][:],
                                     rhs=agg[:, w * 128:(w + 1) * 128],
                                     start=True, stop=True)
                    rT = dpool.tile([64, 128], F32, tag="rT", name="rT")
                    nc.scalar.activation(rT[:], pd[:], AF.Relu,
                                         bias=lb_ts[l][:, 0:1])
                    pt = ps_t.tile([128, 64], F32, tag="pst", name="pst")
                    nc.tensor.transpose(pt[:], rT[:], id64_t[:])
                    nc.scalar.copy(own_nxt[:, w, :], pt[:])

                def finish_block(k):
                    w0 = k * BLKL
                    w1 = min(w0 + BLKL, NW)
                    nb = w1 - w0
                    blk = own_nxt[:, w0:w1, :]
                    mu_s = dpool.tile([128, BLK], F32, tag="mu", name="mu")
                    nc.vector.tensor_reduce(mu_s[:, :nb], blk,
                                            axis=mybir.AxisListType.X, op=OP.add)
                    sq = lnp.tile([128, BLK, D], BF, tag="sq", name="sq")
                    nc.scalar.activation(sq[:, :nb, :], blk, AF.Square)
                    ssq = dpool.tile([128, BLK], F32, tag="ssq", name="ssq")
                    nc.vector.tensor_reduce(ssq[:, :nb], sq[:, :nb, :],
                                            axis=mybir.AxisListType.X, op=OP.add)
                    a2 = dpool.tile([128, BLK], F32, tag="a2", name="a2")
                    nc.vector.tensor_tensor(out=a2[:, :nb], in0=mu_s[:, :nb],
                                            in1=mu_s[:, :nb], op=OP.mult)
                    bvar = dpool.tile([128, BLK], F32, tag="bvar", name="bvar")
                    nc.vector.scalar_tensor_tensor(
                        out=bvar[:, :nb], in0=a2[:, :nb], scalar=-1.0 / D,
                        in1=ssq[:, :nb], op0=OP.mult, op1=OP.add)
                    std = dpool.tile([128, BLK], F32, tag="std", name="std")
                    nc.scalar.activation(std[:, :nb], bvar[:, :nb], AF.Sqrt,
                                         bias=eps_t[:, 0:1], scale=1.0 / D)
                    rstd = dpool.tile([128, BLK], F32, tag="rstd", name="rstd")
                    nc.vector.reciprocal(rstd[:, :nb], std[:, :nb])
                    xc = lnp.tile([128, BLK, D], BF, tag="xc", name="xc")
                    nc.vector.scalar_tensor_tensor(
                        out=xc[:, :nb, :],
                        in0=mu_s[:, :nb, None].to_broadcast([128, nb, D]),
                        scalar=-1.0 / D, in1=blk,
                        op0=OP.mult, op1=OP.add)
                    nc.vector.tensor_tensor(
                        out=blk, in0=xc[:, :nb, :],
                        in1=rstd[:, :nb, None].to_broadcast([128, nb, D]),
                        op=OP.mult)
                    if not ln_trivial:
                        nc.vector.tensor_tensor(
                            out=blk, in0=blk,
                            in1=gm_ts[l][:, None, :].to_broadcast([128, nb, D]),
                            op=OP.mult)
                        nc.vector.tensor_tensor(
                            out=blk, in0=blk,
                            in1=bt_ts[l][:, None, :].to_broadcast([128, nb, D]),
                            op=OP.add)
                    nc.vector.tensor_tensor(out=blk, in0=blk,
                                            in1=own_cur[:, w0:w1, :], op=OP.add)
                    if not last:
                        # write padded bf16 slice rows for the next table
                        stg = stgp.tile([128, BLK, 128], BF, tag="stg", name="stg")
                        nc.scalar.copy(stg[:, :nb, 0:64], blk)
                        so_ap = slice_outs[l].ap().rearrange(
                            "(w p) f -> p w f", p=128)
                        nc.sync.dma_start(out=so_ap[:, w0:w1, :],
                                          in_=stg[:, :nb, :])
                        # early AllGather: windows [0, SPLIT_W) feed buckets
                        # 0..1 of the next layer; fires mid-gather-phase and
                        # overlaps the remaining gathers of this layer
                        if k == ABLK:
                            nc.gpsimd.collective_compute(
                                "AllGather",
                                mybir.AluOpType.bypass,
                                replica_groups=[list(range(C))],
                                ins=[slice_outs[l][0:SPLIT_W * 128, :].opt()],
                                outs=[tabs_lo[l][:].opt()],
                            )
                    else:
                        # final fc on this block, then store to out
                        for w in range(w0, w1):
                            ptf = ps_t.tile([64, 128], F32, tag="pst", name="pst")
                            nc.tensor.transpose(ptf[:], own_nxt[:, w, :],
                                                id128_t[:])
                            hT = dpool.tile([64, 128], BF, tag="hT", name="hT")
                            nc.scalar.copy(hT[:], ptf[:])
                            po = ps_d.tile([64, 128], F32, tag="psd", name="psd")
                            nc.tensor.matmul(po[:], lhsT=fwT_t[:], rhs=hT[:],
                                             start=True, stop=True)
                            ob = dpool.tile([64, 128], F32, tag="ob", name="ob")
                            nc.vector.tensor_scalar_add(ob[:], po[:],
                                                        fb_t[:, 0:1])
                            pq = ps_t.tile([128, 64], F32, tag="pst", name="pst")
                            nc.tensor.transpose(pq[:], ob[:], id64_t[:])
                            nc.scalar.copy(stage[:, w, :], pq[:])
                        nc.sync.dma_start(out=out[:, w0:w1, :],
                                          in_=stage[:, w0:w1, :])

                open_ps = {}
                for ci_, (b, t0, ct) in enumerate(chunks):
                    nidx = ct * 128
                    idx_t = stp.tile([128, CH * 8], I16, tag="idx", name="idx")
                    nc.sync.dma_start(out=idx_t[:, :ct * 8],
                                      in_=idx_w[:, t0 * 8:(t0 + ct) * 8])
                    dst_t = stp.tile([128, CH], BF, tag="dst", name="dst")
                    nc.scalar.dma_start(out=dst_t[:, :ct],
                                        in_=dstloc[:, t0:t0 + ct])
                    w_t = stp.tile([128, CH], BF, tag="w", name="w")
                    nc.scalar.dma_start(out=w_t[:, :ct],
                                        in_=wv[l, :, t0:t0 + ct])

                    gat = gpool.tile([128, CH, 128], BF, tag="gat", name="gat")
                    brows = min(BUCKET, NPAD - b * BUCKET)
                    if l == 0:
                        tab_ap = x_pad[b * BUCKET:b * BUCKET + brows, :]
                    elif b * BUCKET < LO_ROWS:
                        tab_ap = tabs_lo[l - 1][b * BUCKET:b * BUCKET + brows, :]
                    else:
                        r0 = b * BUCKET - LO_ROWS
                        tab_ap = tabs_hi[l - 1][r0:r0 + brows, :]
                    nc.gpsimd.dma_gather(
                        out_ap=gat[:, :ct, :],
                        in_ap=tab_ap,
                        idxs_ap=idx_t[:, :ct * 8],
                        num_idxs=nidx,
                        num_idxs_reg=nidx_regs[nidx],
                        elem_size=128,
                        single_packet=False,
                        queue_num=ci_ % 4,
                    )
                    # scale messages (features 0:64 of each gathered row)
                    msgs = mpool.tile([128, CH, 64], BF, tag="msgs", name="msgs")
                    nc.vector.tensor_tensor(
                        out=msgs[:, :ct, :],
                        in0=gat[:, :ct, 0:64],
                        in1=w_t[:, :ct, None].to_broadcast([128, ct, 64]),
                        op=OP.mult,
                    )
                    oh = opool.tile([128, CH, 128], BF, tag="oh", name="oh")
                    nc.vector.tensor_tensor(
                        out=oh[:, :ct, :],
                        in0=dst_t[:, :ct, None].to_broadcast([128, ct, 128]),
                        in1=iota_t[:].to_broadcast([128, ct, 128]),
                        op=OP.is_equal,
                    )
                    # matmuls per tile
                    for gi, (gb, gw, gt, gstart) in enumerate(groups):
                        if gstart + gt <= t0 or gstart >= t0 + ct:
                            continue
                        lo = max(gstart, t0)
                        hi = min(gstart + gt, t0 + ct)
                        if gstart >= t0:
                            open_ps[gi] = ps_agg.tile([64, 128], F32,
                                                      tag="psagg", name="psagg")
                            if firstgroup.get(gw) == gi:
                                self_write(gw)
                        ps = open_ps[gi]
                        for t in range(lo, hi):
                            ti = t - t0
                            nc.tensor.matmul(
                                ps[:],
                                lhsT=msgs[:, ti, :],
                                rhs=oh[:, ti, :],
                                start=(t == gstart),
                                stop=(t == gstart + gt - 1),
                            )
                        if gstart + gt <= t0 + ct:
                            # group complete: flush into agg
                            nc.vector.tensor_tensor(
                                out=agg[:, gw * 128:(gw + 1) * 128],
                                in0=agg[:, gw * 128:(gw + 1) * 128],
                                in1=ps[:],
                                op=OP.add,
                            )
                            del open_ps[gi]
                            # window complete -> dense; block complete -> LN
                            if lastgroup.get(gw) == gi:
                                dense_window(gw)
                                win_done.add(gw)
                                while (blocks_done[0] < NBLK and all(
                                        w_ in win_done for w_ in
                                        range(blocks_done[0] * BLKL,
                                              min((blocks_done[0] + 1) * BLKL,
                                                  NW)))):
                                    finish_block(blocks_done[0])
                                    blocks_done[0] += 1
                assert not open_ps

                # windows with no groups at all (shouldn't happen, but safe)
                for w in range(NW):
                    if w not in win_done:
                        if w not in self_done:
                            self_write(w)
                        dense_window(w)
                        win_done.add(w)
                while blocks_done[0] < NBLK:
                    finish_block(blocks_done[0])
                    blocks_done[0] += 1

                if not last:
                    nc.gpsimd.collective_compute(
                        "AllGather",
                        mybir.AluOpType.bypass,
                        replica_groups=[list(range(C))],
                        ins=[slice_outs[l][SPLIT_W * 128:SLICE, :].opt()],
                        outs=[tabs_hi[l][:].opt()],
                    )

    if split_waits:
        _split_multi_waits(nc, mybir)
    lower_extended_insts(nc)
    return nc


def kernel(**inputs):
    from concourse.bass_utils import run_bass_kernel_spmd

    x = np.asarray(inputs["x"])
    meta, in_maps = _prep(
        x, np.asarray(inputs["edge_index"]), np.asarray(inputs["edge_attr"]),
        np.asarray(inputs["lin_w"]), np.asarray(inputs["lin_b"]),
        np.asarray(inputs["emlp_w"]), np.asarray(inputs["emlp_b"]),
        np.asarray(inputs["gamma"]), np.asarray(inputs["beta"]),
        np.asarray(inputs["fc_w"]), np.asarray(inputs["fc_b"]))

    key = (meta["NW"], meta["TOT_T"], meta["groups"], meta["chunks"],
           meta["ln_trivial"])
    if key not in _CACHE:
        _CACHE[key] = _build(meta)
    nc = _CACHE[key]

    res = run_bass_kernel_spmd(nc, in_maps, list(range(C)))
    N = meta["N"]
    NW = meta["NW"]
    parts = []
    for c in range(C):
        o = np.asarray(res.results[c]["out"])  # [128, NW, 64]
        parts.append(np.transpose(o, (1, 0, 2)).reshape(NW * 128, D))
    full = np.concatenate(parts, axis=0)[:N]
    return full.astype(np.float32)


# revision 34
# speedup vs baseline: 1.1259x; 1.0075x over previous
"""Trainium2 Bass kernel for the EnhancedGNNEncoder (3-layer HydroConv GNN).

Strategy (8 NeuronCores, SPMD):
  - Nodes range-partitioned across cores (dst-sharding). Each core aggregates
    messages for its own nodes, computes the dense update for its slice, and
    an AllGather rebuilds the full node table for the next layer's gathers.
  - The node table is stored bf16 padded to 128 features per row (256 B rows,
    the dma_gather minimum element size), so gathers land directly in bf16
    and the per-edge weight multiply runs in place on the gathered tile.
  - The dst-gather of the reference (w * (h[src] - h[dst])) is eliminated
    algebraically: agg[n] = sum_e w_e h[src_e] - wdeg[n] h[n]. The second
    term is computed on-chip from the resident own-slice (scale by -wdeg,
    transpose on the tensor engine, add into agg) instead of as gathered
    self-edges -- saving ~3% of gather descriptors.
  - Per-edge weights w_e = softplus(edge_attr @ emlp_w + emlp_b) depend only
    on inputs, so they are computed host-side and streamed per-edge.
  - The dense phase (linear + relu), block LayerNorm + residual, the final
    fc, and the slice writeback are all interleaved into the gather phase:
    buckets are ordered so the largest bucket runs last, and each window's
    dense update fires as soon as its final group is flushed. This keeps the
    GpSimd engine (descriptor generation -- the bottleneck) streaming with
    minimal idle at layer boundaries.

The instruction stream is identical on all cores (SPMD); per-core variation
lives in the input tensors. Per-(bucket,window) tile counts are max-reduced
over cores and padded with null edges (w=0).
"""

import math

import numpy as np

D = 64
L = 3
C = 8
WIN = 128
BUCKET = 28672
EPS = 1e-5
CH = 32       # gather-chunk size in 128-edge tiles
BLK = 14      # windows per LayerNorm/writeback block
BLK_LAST = 7  # smaller blocks in the last layer shrink the kernel tail
SPLIT_W = 56  # windows in the first (early) AllGather = buckets 0-1

_CACHE = {}


def _softplus(z):
    return np.logaddexp(0.0, z)


def _prep(x, edge_index, edge_attr, lin_w, lin_b, emlp_w, emlp_b, gamma, beta,
          fc_w, fc_b):
    import ml_dtypes
    BF = ml_dtypes.bfloat16

    N = x.shape[0]
    E = edge_index.shape[1]
    NW = math.ceil(N / (C * WIN))
    SLICE = NW * WIN
    NPAD = C * SLICE
    NB = math.ceil(NPAD / BUCKET)

    src = np.ascontiguousarray(edge_index[0]).astype(np.int64)
    dst = np.ascontiguousarray(edge_index[1]).astype(np.int64)
    ea = np.asarray(edge_attr, dtype=np.float32)

    # per-layer edge weights + per-node weighted degree
    w_layers = np.empty((L, E), dtype=np.float32)
    wdeg = np.empty((L, NPAD), dtype=np.float32)
    for l in range(L):
        z = ea @ np.asarray(emlp_w[l, 0], dtype=np.float32) + float(emlp_b[l, 0])
        w_layers[l] = _softplus(z).astype(np.float32)
        wdeg[l] = np.bincount(dst, weights=w_layers[l].astype(np.float64),
                              minlength=NPAD).astype(np.float32)
    # negated, per-core [128, L, NW] layout (node = c*SLICE + w*128 + p)
    nwdeg = (-wdeg).reshape(L, C, NW, WIN)
    nwdeg = np.transpose(nwdeg, (1, 3, 0, 2)).copy()  # [C, 128, L, NW]

    core_of = dst // SLICE

    # split global renumbering: the node table is the concatenation of a "lo"
    # half (each core's windows [0, SPLIT_W), rank-major) and a "hi" half
    # (windows [SPLIT_W, NW), rank-major). Each half is produced by ONE
    # contiguous AllGather; lo = gather buckets 0..1 fires early (mid-layer),
    # so the next layer's first buckets depend only on it.
    n_all = np.arange(NPAD, dtype=np.int64)
    c_all = n_all // SLICE
    loc = n_all % SLICE
    w_all = loc // WIN
    p_all = loc % WIN
    LO_ROWS = SPLIT_W * C * WIN
    gid_of = np.where(
        w_all < SPLIT_W,
        c_all * (SPLIT_W * WIN) + w_all * WIN + p_all,
        LO_ROWS + c_all * ((NW - SPLIT_W) * WIN) + (w_all - SPLIT_W) * WIN + p_all)
    src_g = gid_of[src]

    per_core = []
    counts = np.zeros((C, NB, NW), dtype=np.int64)
    for c in range(C):
        m = core_of == c
        s_c = src_g[m]
        d_c = dst[m]
        w_c = w_layers[:, m]
        b_c = s_c // BUCKET
        wl_c = (d_c - c * SLICE) // WIN
        order = np.lexsort((wl_c, b_c))
        s_c, d_c, w_c = s_c[order], d_c[order], w_c[:, order]
        b_c, wl_c = b_c[order], wl_c[order]
        np.add.at(counts[c], (b_c, wl_c), 1)
        per_core.append((s_c, d_c, w_c, b_c, wl_c))

    maxcnt = counts.max(axis=0)  # [NB, NW]
    tiles = np.where(maxcnt > 0, (maxcnt + 127) // 128, 0).astype(np.int64)
    # bucket order: early-collective buckets (0,1) first, largest bucket last
    bucket_tiles = tiles.sum(axis=1)
    early = [b for b in range(NB) if (b + 1) * BUCKET <= SPLIT_W * C * WIN]
    rest = sorted((b for b in range(NB) if b not in early),
                  key=lambda b: (bucket_tiles[b], b))
    border = early + rest
    # group schedule shared across cores
    groups = []  # (bucket, window, n_tiles, tile_start)
    tpos = 0
    for b in border:
        for w in range(NW):
            t = int(tiles[b, w])
            if t == 0:
                continue
            groups.append((b, w, t, tpos))
            tpos += t
    TOT_T = tpos
    TOT = TOT_T * 128

    # last group index per window (dense fires after this group's flush);
    # first group index per window (self-term write issued when it opens)
    lastgroup = {}
    firstgroup = {}
    for gi, (b, w, t, ts) in enumerate(groups):
        lastgroup[w] = gi
        if w not in firstgroup:
            firstgroup[w] = gi

    # fill per-core streams
    idx16 = np.zeros((C, TOT), dtype=np.int16)
    dstloc = np.full((C, TOT), -1.0, dtype=np.float32)
    wvals = np.zeros((C, L, TOT), dtype=np.float32)
    for c in range(C):
        s_c, d_c, w_c, b_c, wl_c = per_core[c]
        # edges sorted by (b, w) lexicographic; groups are in border order
        starts = {}
        epos = 0
        for b in range(NB):
            for w in range(NW):
                n = int(counts[c, b, w])
                starts[(b, w)] = (epos, n)
                epos += n
        assert epos == len(s_c)
        for (b, w, t, tstart) in groups:
            epos, n = starts[(b, w)]
            if n:
                sl = slice(epos, epos + n)
                o = tstart * 128
                idx16[c, o:o + n] = (s_c[sl] - b * BUCKET).astype(np.int16)
                dstloc[c, o:o + n] = (d_c[sl] - (c * SLICE + w * WIN)).astype(np.float32)
                wvals[c, :, o:o + n] = w_c[:, sl]

    # chunks: consecutive tile runs within one bucket (in border order);
    # the final tiles of the last bucket are tapered into small chunks so the
    # kernel tail (SDMA drain of the last chunk + dense of the last windows)
    # stays short
    chunks = []  # (bucket, tile_start, n_tiles)
    for b in border:
        bt = [g for g in groups if g[0] == b]
        if not bt:
            continue
        b0 = bt[0][3]
        bn = bt[-1][3] + bt[-1][2]
        t = b0
        while t < bn:
            ct = min(CH, bn - t)
            if b == border[-1] and bn - t <= CH:
                ct = min(8, bn - t)
            chunks.append((b, t, ct))
            t += ct

    # mark chunk-trailing pad slots with idx -1: the Q7 desc-gen kernel trims
    # trailing negatives per call, skipping their descriptors on cores with
    # fewer real edges (interior pads stay 0 -- gathered then zeroed by w=0)
    pad = dstloc < 0  # [C, TOT]  (trim disabled for bisect)

    # device layouts
    # wrapped gather indices: edge i -> [i % 16, i // 16], replicated x8
    idx_wrapped = np.zeros((C, 128, TOT // 16), dtype=np.int16)
    for c in range(C):
        w16 = idx16[c].reshape(TOT // 16, 16).T  # [16, TOT//16]
        idx_wrapped[c] = np.tile(w16, (8, 1))
    # per-tile-major: [128, TOT_T]: (p, t) = edge t*128+p
    dstloc_t = np.transpose(dstloc.reshape(C, TOT_T, 128), (0, 2, 1)).astype(BF)
    wvals_t = np.transpose(wvals.reshape(C, L, TOT_T, 128), (0, 1, 3, 2)).astype(BF)

    # node table: bf16 padded to 128 features (256B rows), rows in gid order
    x_pad = np.zeros((NPAD, 128), dtype=BF)
    x_pad[gid_of[:N], :D] = np.asarray(x, dtype=np.float32).astype(BF)
    x_f32 = np.zeros((NPAD, D), dtype=np.float32)
    x_f32[:N] = np.asarray(x, dtype=np.float32)
    x_own = np.transpose(
        x_f32.reshape(C, NW, 128, D), (0, 2, 1, 3)).copy()  # [C, 128, NW, 64]

    iota = np.broadcast_to(np.arange(128, dtype=np.float32), (128, 1, 128)).astype(BF)
    id64 = np.eye(64, dtype=np.float32)
    id128 = np.eye(128, dtype=np.float32)
    lwT = np.transpose(np.asarray(lin_w, dtype=np.float32), (0, 2, 1)).astype(BF).copy()
    fwT = np.asarray(fc_w, dtype=np.float32).T.astype(BF).copy()

    gamma = np.asarray(gamma, dtype=np.float32)
    beta = np.asarray(beta, dtype=np.float32)
    ln_trivial = bool(np.all(gamma == 1.0) and np.all(beta == 0.0))

    meta = dict(N=N, NW=NW, SLICE=SLICE, NPAD=NPAD, NB=NB, TOT_T=TOT_T,
                groups=tuple(groups), chunks=tuple(chunks),
                lastgroup=tuple(sorted(lastgroup.items())),
                firstgroup=tuple(sorted(firstgroup.items())),
                ln_trivial=ln_trivial)

    in_maps = []
    for c in range(C):
        in_maps.append({
            "x_pad": x_pad,
            "x_own": x_own[c],
            "idx_w": idx_wrapped[c],
            "dstloc": dstloc_t[c],
            "wv": wvals_t[c],
            "nwdeg": nwdeg[c],
            "iota": iota,
            "id64": id64,
            "id128": id128,
            "lwT": lwT,
            "lb": np.asarray(lin_b, dtype=np.float32),
            "fwT": fwT,
            "fb": np.asarray(fc_b, dtype=np.float32).reshape(64, 1),
            "gm": np.broadcast_to(gamma[:, None, :], (L, 128, D)).copy(),
            "bt": np.broadcast_to(beta[:, None, :], (L, 128, D)).copy(),
        })
    return meta, in_maps


def _split_multi_waits(nc, mybir):
    """This walrus build rejects >1 sync-wait per instruction; hoist extras
    onto single-wait NOPs inserted just before, same engine."""
    ctr = 0
    for bbw in nc.bb_map.values():
        bb = bbw.bb
        insts = bb.instructions
        new = []
        changed = False
        for inst in insts:
            si = inst.sync_info
            waits = list(si.on_wait) if si and si.on_wait else []
            if len(waits) > 1:
                changed = True
                for w in waits[:-1]:
                    ctr += 1
                    new.append(mybir.InstNoOp(
                        name=f"I-waitsplit-{ctr}",
                        engine=inst.engine,
                        sync_info=mybir.SyncInfo(on_wait=[w], on_update=[]),
                    ))
                si.on_wait = [waits[-1]]
            new.append(inst)
        if changed:
            bb.instructions = new
    return nc


def _build(meta, split_waits=True, n_layers=L):
    import concourse.bass as bass
    import concourse.mybir as mybir
    from concourse import library_config
    from concourse.library_overlay import lower_extended_insts
    from concourse.tile import TileContext

    NW = meta["NW"]
    SLICE = meta["SLICE"]
    NPAD = meta["NPAD"]
    NB = meta["NB"]
    TOT_T = meta["TOT_T"]
    groups = meta["groups"]
    chunks = meta["chunks"]
    lastgroup = dict(meta["lastgroup"])
    firstgroup = dict(meta["firstgroup"])
    ln_trivial = meta["ln_trivial"]
    TOT = TOT_T * 128
    # block index after which the early AllGather (windows < SPLIT_W) fires
    ABLK = math.ceil(SPLIT_W / BLK) - 1

    F32 = mybir.dt.float32
    BF = mybir.dt.bfloat16
    I16 = mybir.dt.int16
    AF = mybir.ActivationFunctionType
    OP = mybir.AluOpType

    nc = bass.Bass(num_devices=C, num_swdge_queues=4)

    x_pad = nc.declare_dram_parameter("x_pad", [NPAD, 128], BF, isOutput=False)
    x_own = nc.declare_dram_parameter("x_own", [128, NW, D], F32, isOutput=False)
    idx_w = nc.declare_dram_parameter("idx_w", [128, TOT // 16], I16, isOutput=False)
    dstloc = nc.declare_dram_parameter("dstloc", [128, TOT_T], BF, isOutput=False)
    wv = nc.declare_dram_parameter("wv", [L, 128, TOT_T], BF, isOutput=False)
    nwdeg = nc.declare_dram_parameter("nwdeg", [128, L, NW], F32, isOutput=False)
    iota = nc.declare_dram_parameter("iota", [128, 1, 128], BF, isOutput=False)
    id64 = nc.declare_dram_parameter("id64", [64, 64], F32, isOutput=False)
    id128 = nc.declare_dram_parameter("id128", [128, 128], F32, isOutput=False)
    lwT = nc.declare_dram_parameter("lwT", [L, 64, 64], BF, isOutput=False)
    lb = nc.declare_dram_parameter("lb", [L, 64], F32, isOutput=False)
    fwT = nc.declare_dram_parameter("fwT", [64, 64], BF, isOutput=False)
    fb = nc.declare_dram_parameter("fb", [64, 1], F32, isOutput=False)
    if not ln_trivial:
        gm = nc.declare_dram_parameter("gm", [L, 128, 64], F32, isOutput=False)
        bt = nc.declare_dram_parameter("bt", [L, 128, 64], F32, isOutput=False)
    out = nc.declare_dram_parameter("out", [128, NW, D], F32, isOutput=True)

    LO_ROWS = SPLIT_W * C * WIN
    HI_ROWS = NPAD - LO_ROWS
    if n_layers > 1:
        tabs_lo = [
            nc.dram_tensor("tabA_lo", [LO_ROWS, 128], BF, addr_space="Shared"),
            nc.dram_tensor("tabB_lo", [LO_ROWS, 128], BF, addr_space="Shared"),
        ]
        tabs_hi = [
            nc.dram_tensor("tabA_hi", [HI_ROWS, 128], BF, addr_space="Shared"),
            nc.dram_tensor("tabB_hi", [HI_ROWS, 128], BF, addr_space="Shared"),
        ]
        slice_outs = [
            nc.dram_tensor("slice0", [SLICE, 128], BF),
            nc.dram_tensor("slice1", [SLICE, 128], BF),
        ]
    else:
        tabs_lo, tabs_hi, slice_outs = [x_pad, x_pad], [x_pad, x_pad], []

    nc.gpsimd.load_library(library_config.mlp)

    with TileContext(nc) as tc:
        with (
            tc.tile_pool(name="const", bufs=1) as cpool,
            tc.tile_pool(name="big", bufs=1) as bigp,
            tc.tile_pool(name="gat", bufs=5) as gpool,
            tc.tile_pool(name="msg", bufs=4) as mpool,
            tc.tile_pool(name="oh", bufs=3) as opool,
            tc.tile_pool(name="strm", bufs=8) as stp,
            tc.tile_pool(name="dense", bufs=3) as dpool,
            tc.tile_pool(name="lnp", bufs=1) as lnp,
            tc.tile_pool(name="stgp", bufs=2) as stgp,
            tc.tile_pool(name="psagg", bufs=3, space="PSUM") as ps_agg,
            tc.tile_pool(name="psd", bufs=2, space="PSUM") as ps_d,
            tc.tile_pool(name="pst", bufs=3, space="PSUM") as ps_t,
        ):
            # constants
            iota_t = cpool.tile([128, 1, 128], BF)
            nc.sync.dma_start(out=iota_t[:], in_=iota[:, :, :])
            id64_t = cpool.tile([64, 64], F32)
            nc.sync.dma_start(out=id64_t[:], in_=id64[:, :])
            id128_t = cpool.tile([128, 128], F32)
            nc.sync.dma_start(out=id128_t[:], in_=id128[:, :])
            nwdeg_t = cpool.tile([128, L, NW], F32)
            nc.sync.dma_start(out=nwdeg_t[:], in_=nwdeg[:, :, :])
            lwT_ts = []
            for l in range(L):
                t = cpool.tile([64, 64], BF, tag=f"lwT{l}")
                nc.sync.dma_start(out=t[:], in_=lwT[l, :, :])
                lwT_ts.append(t)
            lb_ts = []
            for l in range(L):
                t = cpool.tile([64, 1], F32, tag=f"lb{l}")
                nc.sync.dma_start(out=t[:], in_=lb[l, :, None])
                lb_ts.append(t)
            fwT_t = cpool.tile([64, 64], BF)
            nc.sync.dma_start(out=fwT_t[:], in_=fwT[:, :])
            fb_t = cpool.tile([64, 1], F32)
            nc.sync.dma_start(out=fb_t[:], in_=fb[:, :])
            gm_ts, bt_ts = [], []
            if not ln_trivial:
                for l in range(L):
                    g_ = cpool.tile([128, 64], F32, tag=f"gm{l}")
                    nc.sync.dma_start(out=g_[:], in_=gm[l, :, :])
                    gm_ts.append(g_)
                    b_ = cpool.tile([128, 64], F32, tag=f"bt{l}")
                    nc.sync.dma_start(out=b_[:], in_=bt[l, :, :])
                    bt_ts.append(b_)

            eps_t = cpool.tile([128, 1], F32)
            nc.vector.memset(eps_t[:], EPS)
            # one register per distinct gather size, reused across all calls
            nidx_regs = {}
            for (_b, _t0, _ct) in chunks:
                v = _ct * 128
                if v not in nidx_regs:
                    nidx_regs[v] = nc.gpsimd.to_reg(v)

            own = [bigp.tile([128, NW, D], F32, tag="own_a", name="own_a"),
                   bigp.tile([128, NW, D], F32, tag="own_b", name="own_b")]
            nc.sync.dma_start(out=own[0][:], in_=x_own[:, :, :])
            agg = bigp.tile([64, NW * 128], BF, tag="agg", name="agg")
            stage = bigp.tile([128, NW, D], F32, tag="stage", name="stage")

            for l in range(n_layers):
                own_cur = own[l % 2]
                own_nxt = own[(l + 1) % 2]
                last = l == n_layers - 1

                BLKL = BLK_LAST if l == n_layers - 1 else BLK
                NBLK = math.ceil(NW / BLKL)
                win_done = set()
                self_done = set()
                blocks_done = [0]

                def self_write(w):
                    # agg[:, w] = transpose(own_cur[:, w, :] * -wdeg)
                    # (first write of the window; flushes add on top)
                    sc = dpool.tile([128, 64], F32, tag="sc", name="sc")
                    nc.vector.tensor_tensor(
                        out=sc[:], in0=own_cur[:, w, :],
                        in1=nwdeg_t[:, l, w, None].to_broadcast([128, 64]),
                        op=OP.mult)
                    pt = ps_t.tile([64, 128], F32, tag="pst", name="pst")
                    nc.tensor.transpose(pt[:], sc[:], id128_t[:])
                    nc.scalar.copy(agg[:, w * 128:(w + 1) * 128], pt[:])
                    self_done.add(w)

                def dense_window(w):
                    pd = ps_d.tile([64, 128], F32, tag="psd", name="psd")
                    nc.tensor.matmul(pd[:], lhsT=lwT_ts[l][:],
                                     rhs=agg[:, w * 128:(w + 1) * 128],
                                     start=True, stop=True)
                    rT = dpool.tile([64, 128], F32, tag="rT", name="rT")
                    nc.scalar.activation(rT[:], pd[:], AF.Relu,
                                         bias=lb_ts[l][:, 0:1])
                    pt = ps_t.tile([128, 64], F32, tag="pst", name="pst")
                    nc.tensor.transpose(pt[:], rT[:], id64_t[:])
                    nc.scalar.copy(own_nxt[:, w, :], pt[:])

                def finish_block(k):
                    w0 = k * BLKL
                    w1 = min(w0 + BLKL, NW)
                    nb = w1 - w0
                    blk = own_nxt[:, w0:w1, :]
                    mu_s = dpool.tile([128, BLK], F32, tag="mu", name="mu")
                    nc.vector.tensor_reduce(mu_s[:, :nb], blk,
                                            axis=mybir.AxisListType.X, op=OP.add)
                    sq = lnp.tile([128, BLK, D], BF, tag="sq", name="sq")
                    nc.scalar.activation(sq[:, :nb, :], blk, AF.Square)
                    ssq = dpool.tile([128, BLK], F32, tag="ssq", name="ssq")
                    nc.vector.tensor_reduce(ssq[:, :nb], sq[:, :nb, :],
                                            axis=mybir.AxisListType.X, op=OP.add)
                    a2 = dpool.tile([128, BLK], F32, tag="a2", name="a2")
                    nc.vector.tensor_tensor(out=a2[:, :nb], in0=mu_s[:, :nb],
                                            in1=mu_s[:, :nb], op=OP.mult)
                    bvar = dpool.tile([128, BLK], F32, tag="bvar", name="bvar")
                    nc.vector.scalar_tensor_tensor(
                        out=bvar[:, :nb], in0=a2[:, :nb], scalar=-1.0 / D,
                        in1=ssq[:, :nb], op0=OP.mult, op1=OP.add)
                    std = dpool.tile([128, BLK], F32, tag="std", name="std")
                    nc.scalar.activation(std[:, :nb], bvar[:, :nb], AF.Sqrt,
                                         bias=eps_t[:, 0:1], scale=1.0 / D)
                    rstd = dpool.tile([128, BLK], F32, tag="rstd", name="rstd")
                    nc.vector.reciprocal(rstd[:, :nb], std[:, :nb])
                    xc = lnp.tile([128, BLK, D], BF, tag="xc", name="xc")
                    nc.vector.scalar_tensor_tensor(
                        out=xc[:, :nb, :],
                        in0=mu_s[:, :nb, None].to_broadcast([128, nb, D]),
                        scalar=-1.0 / D, in1=blk,
                        op0=OP.mult, op1=OP.add)
                    nc.vector.tensor_tensor(
                        out=blk, in0=xc[:, :nb, :],
                        in1=rstd[:, :nb, None].to_broadcast([128, nb, D]),
                        op=OP.mult)
                    if not ln_trivial:
                        nc.vector.tensor_tensor(
                            out=blk, in0=blk,
                            in1=gm_ts[l][:, None, :].to_broadcast([128, nb, D]),
                            op=OP.mult)
                        nc.vector.tensor_tensor(
                            out=blk, in0=blk,
                            in1=bt_ts[l][:, None, :].to_broadcast([128, nb, D]),
                            op=OP.add)
                    nc.vector.tensor_tensor(out=blk, in0=blk,
                                            in1=own_cur[:, w0:w1, :], op=OP.add)
                    if not last:
                        # write padded bf16 slice rows for the next table
                        stg = stgp.tile([128, BLK, 128], BF, tag="stg", name="stg")
                        nc.scalar.copy(stg[:, :nb, 0:64], blk)
                        so_ap = slice_outs[l].ap().rearrange(
                            "(w p) f -> p w f", p=128)
                        nc.sync.dma_start(out=so_ap[:, w0:w1, :],
                                          in_=stg[:, :nb, :])
                        # early AllGather: windows [0, SPLIT_W) feed buckets
                        # 0..1 of the next layer; fires mid-gather-phase and
                        # overlaps the remaining gathers of this layer
                        if k == ABLK:
                            nc.gpsimd.collective_compute(
                                "AllGather",
                                mybir.AluOpType.bypass,
                                replica_groups=[list(range(C))],
                                ins=[slice_outs[l][0:SPLIT_W * 128, :].opt()],
                                outs=[tabs_lo[l][:].opt()],
                            )
                    else:
                        # final fc on this block, then store to out
                        for w in range(w0, w1):
                            ptf = ps_t.tile([64, 128], F32, tag="pst", name="pst")
                            nc.tensor.transpose(ptf[:], own_nxt[:, w, :],
                                                id128_t[:])
                            hT = dpool.tile([64, 128], BF, tag="hT", name="hT")
                            nc.scalar.copy(hT[:], ptf[:])
                            po = ps_d.tile([64, 128], F32, tag="psd", name="psd")
                            nc.tensor.matmul(po[:], lhsT=fwT_t[:], rhs=hT[:],
                                             start=True, stop=True)
                            ob = dpool.tile([64, 128], F32, tag="ob", name="ob")
                            nc.vector.tensor_scalar_add(ob[:], po[:],
                                                        fb_t[:, 0:1])
                            pq = ps_t.tile([128, 64], F32, tag="pst", name="pst")
                            nc.tensor.transpose(pq[:], ob[:], id64_t[:])
                            nc.scalar.copy(stage[:, w, :], pq[:])
                        nc.sync.dma_start(out=out[:, w0:w1, :],
                                          in_=stage[:, w0:w1, :])

                open_ps = {}
                for ci_, (b, t0, ct) in enumerate(chunks):
                    nidx = ct * 128
                    idx_t = stp.tile([128, CH * 8], I16, tag="idx", name="idx")
                    nc.sync.dma_start(out=idx_t[:, :ct * 8],
                                      in_=idx_w[:, t0 * 8:(t0 + ct) * 8])
                    dst_t = stp.tile([128, CH], BF, tag="dst", name="dst")
                    nc.scalar.dma_start(out=dst_t[:, :ct],
                                        in_=dstloc[:, t0:t0 + ct])
                    w_t = stp.tile([128, CH], BF, tag="w", name="w")
                    nc.scalar.dma_start(out=w_t[:, :ct],
                                        in_=wv[l, :, t0:t0 + ct])

                    gat = gpool.tile([128, CH, 128], BF, tag="gat", name="gat")
                    brows = min(BUCKET, NPAD - b * BUCKET)
                    if l == 0:
                        tab_ap = x_pad[b * BUCKET:b * BUCKET + brows, :]
                    elif b * BUCKET < LO_ROWS:
                        tab_ap = tabs_lo[l - 1][b * BUCKET:b * BUCKET + brows, :]
                    else:
                        r0 = b * BUCKET - LO_ROWS
                        tab_ap = tabs_hi[l - 1][r0:r0 + brows, :]
                    nc.gpsimd.dma_gather(
                        out_ap=gat[:, :ct, :],
                        in_ap=tab_ap,
                        idxs_ap=idx_t[:, :ct * 8],
                        num_idxs=nidx,
                        num_idxs_reg=nidx_regs[nidx],
                        elem_size=128,
                        single_packet=False,
                        queue_num=ci_ % 4,
                    )
                    # scale messages (features 0:64 of each gathered row)
                    msgs = mpool.tile([128, CH, 64], BF, tag="msgs", name="msgs")
                    nc.vector.tensor_tensor(
                        out=msgs[:, :ct, :],
                        in0=gat[:, :ct, 0:64],
                        in1=w_t[:, :ct, None].to_broadcast([128, ct, 64]),
                        op=OP.mult,
                    )
                    oh = opool.tile([128, CH, 128], BF, tag="oh", name="oh")
                    nc.vector.tensor_tensor(
                        out=oh[:, :ct, :],
                        in0=dst_t[:, :ct, None].to_broadcast([128, ct, 128]),
                        in1=iota_t[:].to_broadcast([128, ct, 128]),
                        op=OP.is_equal,
                    )
                    # matmuls per tile
                    for gi, (gb, gw, gt, gstart) in enumerate(groups):
                        if gstart + gt <= t0 or gstart >= t0 + ct:
                            continue
                        lo = max(gstart, t0)
                        hi = min(gstart + gt, t0 + ct)
                        if gstart >= t0:
                            open_ps[gi] = ps_agg.tile([64, 128], F32,
                                                      tag="psagg", name="psagg")
                            if firstgroup.get(gw) == gi:
                                self_write(gw)
                        ps = open_ps[gi]
                        for t in range(lo, hi):
                            ti = t - t0
                            nc.tensor.matmul(
                                ps[:],
                                lhsT=msgs[:, ti, :],
                                rhs=oh[:, ti, :],
                                start=(t == gstart),
                                stop=(t == gstart + gt - 1),
                            )
                        if gstart + gt <= t0 + ct:
                            # group complete: flush into agg
                            nc.vector.tensor_tensor(
                                out=agg[:, gw * 128:(gw + 1) * 128],
                                in0=agg[:, gw * 128:(gw + 1) * 128],
                                in1=ps[:],
                                op=OP.add,
                            )
                            del open_ps[gi]
                            # window complete -> dense; block complete -> LN
                            if lastgroup.get(gw) == gi:
                                dense_window(gw)
                                win_done.add(gw)
                                while (blocks_done[0] < NBLK and all(
                                        w_ in win_done for w_ in
                                        range(blocks_done[0] * BLKL,
                                              min((blocks_done[0] + 1) * BLKL,
                                                  NW)))):
                                    finish_block(blocks_done[0])
                                    blocks_done[0] += 1
                assert not open_ps

                # windows with no groups at all (shouldn't happen, but safe)
                for w in range(NW):
                    if w not in win_done:
                        if w not in self_done:
                            self_write(w)
                        dense_window(w)
                        win_done.add(w)
                while blocks_done[0] < NBLK:
                    finish_block(blocks_done[0])
                    blocks_done[0] += 1

                if not last:
                    nc.gpsimd.collective_compute(
                        "AllGather",
                        mybir.AluOpType.bypass,
                        replica_groups=[list(range(C))],
                        ins=[slice_outs[l][SPLIT_W * 128:SLICE, :].opt()],
                        outs=[tabs_hi[l][:].opt()],
                    )

    if split_waits:
        _split_multi_waits(nc, mybir)
    lower_extended_insts(nc)
    return nc


def kernel(**inputs):
    from concourse.bass_utils import run_bass_kernel_spmd

    x = np.asarray(inputs["x"])
    meta, in_maps = _prep(
        x, np.asarray(inputs["edge_index"]), np.asarray(inputs["edge_attr"]),
        np.asarray(inputs["lin_w"]), np.asarray(inputs["lin_b"]),
        np.asarray(inputs["emlp_w"]), np.asarray(inputs["emlp_b"]),
        np.asarray(inputs["gamma"]), np.asarray(inputs["beta"]),
        np.asarray(inputs["fc_w"]), np.asarray(inputs["fc_b"]))

    key = (meta["NW"], meta["TOT_T"], meta["groups"], meta["chunks"],
           meta["ln_trivial"])
    if key not in _CACHE:
        _CACHE[key] = _build(meta)
    nc = _CACHE[key]

    res = run_bass_kernel_spmd(nc, in_maps, list(range(C)))
    N = meta["N"]
    NW = meta["NW"]
    parts = []
    for c in range(C):
        o = np.asarray(res.results[c]["out"])  # [128, NW, 64]
        parts.append(np.transpose(o, (1, 0, 2)).reshape(NW * 128, D))
    full = np.concatenate(parts, axis=0)[:N]
    return full.astype(np.float32)


# revision 35
# speedup vs baseline: 1.1322x; 1.0057x over previous
"""Trainium2 Bass kernel for the EnhancedGNNEncoder (3-layer HydroConv GNN).

Strategy (8 NeuronCores, SPMD):
  - Nodes range-partitioned across cores (dst-sharding). Each core aggregates
    messages for its own nodes, computes the dense update for its slice, and
    an AllGather rebuilds the full node table for the next layer's gathers.
  - The node table is stored bf16 padded to 128 features per row (256 B rows,
    the dma_gather minimum element size), so gathers land directly in bf16
    and the per-edge weight multiply runs in place on the gathered tile.
  - The dst-gather of the reference (w * (h[src] - h[dst])) is eliminated
    algebraically: agg[n] = sum_e w_e h[src_e] - wdeg[n] h[n]. The second
    term is computed on-chip from the resident own-slice (scale by -wdeg,
    transpose on the tensor engine, add into agg) instead of as gathered
    self-edges -- saving ~3% of gather descriptors.
  - Per-edge weights w_e = softplus(edge_attr @ emlp_w + emlp_b) depend only
    on inputs, so they are computed host-side and streamed per-edge.
  - The dense phase (linear + relu), block LayerNorm + residual, the final
    fc, and the slice writeback are all interleaved into the gather phase:
    buckets are ordered so the largest bucket runs last, and each window's
    dense update fires as soon as its final group is flushed. This keeps the
    GpSimd engine (descriptor generation -- the bottleneck) streaming with
    minimal idle at layer boundaries.

The instruction stream is identical on all cores (SPMD); per-core variation
lives in the input tensors. Per-(bucket,window) tile counts are max-reduced
over cores and padded with null edges (w=0).
"""

import math

import numpy as np

D = 64
L = 3
C = 8
WIN = 128
BUCKET = 28672
EPS = 1e-5
CH = 31       # gather-chunk tiles; 31*128/16+1=249 descs/lane fits the 256-entry ring
BLK = 14      # windows per LayerNorm/writeback block
BLK_LAST = 7  # smaller blocks in the last layer shrink the kernel tail
SPLIT_W = 56  # windows in the first (early) AllGather = buckets 0-1

_CACHE = {}


def _softplus(z):
    return np.logaddexp(0.0, z)


def _prep(x, edge_index, edge_attr, lin_w, lin_b, emlp_w, emlp_b, gamma, beta,
          fc_w, fc_b):
    import ml_dtypes
    BF = ml_dtypes.bfloat16

    N = x.shape[0]
    E = edge_index.shape[1]
    NW = math.ceil(N / (C * WIN))
    SLICE = NW * WIN
    NPAD = C * SLICE
    NB = math.ceil(NPAD / BUCKET)

    src = np.ascontiguousarray(edge_index[0]).astype(np.int64)
    dst = np.ascontiguousarray(edge_index[1]).astype(np.int64)
    ea = np.asarray(edge_attr, dtype=np.float32)

    # per-layer edge weights + per-node weighted degree
    w_layers = np.empty((L, E), dtype=np.float32)
    wdeg = np.empty((L, NPAD), dtype=np.float32)
    for l in range(L):
        z = ea @ np.asarray(emlp_w[l, 0], dtype=np.float32) + float(emlp_b[l, 0])
        w_layers[l] = _softplus(z).astype(np.float32)
        wdeg[l] = np.bincount(dst, weights=w_layers[l].astype(np.float64),
                              minlength=NPAD).astype(np.float32)
    # negated, per-core [128, L, NW] layout (node = c*SLICE + w*128 + p)
    nwdeg = (-wdeg).reshape(L, C, NW, WIN)
    nwdeg = np.transpose(nwdeg, (1, 3, 0, 2)).copy()  # [C, 128, L, NW]

    core_of = dst // SLICE

    # split global renumbering: the node table is the concatenation of a "lo"
    # half (each core's windows [0, SPLIT_W), rank-major) and a "hi" half
    # (windows [SPLIT_W, NW), rank-major). Each half is produced by ONE
    # contiguous AllGather; lo = gather buckets 0..1 fires early (mid-layer),
    # so the next layer's first buckets depend only on it.
    n_all = np.arange(NPAD, dtype=np.int64)
    c_all = n_all // SLICE
    loc = n_all % SLICE
    w_all = loc // WIN
    p_all = loc % WIN
    LO_ROWS = SPLIT_W * C * WIN
    gid_of = np.where(
        w_all < SPLIT_W,
        c_all * (SPLIT_W * WIN) + w_all * WIN + p_all,
        LO_ROWS + c_all * ((NW - SPLIT_W) * WIN) + (w_all - SPLIT_W) * WIN + p_all)
    src_g = gid_of[src]

    per_core = []
    counts = np.zeros((C, NB, NW), dtype=np.int64)
    for c in range(C):
        m = core_of == c
        s_c = src_g[m]
        d_c = dst[m]
        w_c = w_layers[:, m]
        b_c = s_c // BUCKET
        wl_c = (d_c - c * SLICE) // WIN
        order = np.lexsort((wl_c, b_c))
        s_c, d_c, w_c = s_c[order], d_c[order], w_c[:, order]
        b_c, wl_c = b_c[order], wl_c[order]
        np.add.at(counts[c], (b_c, wl_c), 1)
        per_core.append((s_c, d_c, w_c, b_c, wl_c))

    maxcnt = counts.max(axis=0)  # [NB, NW]
    tiles = np.where(maxcnt > 0, (maxcnt + 127) // 128, 0).astype(np.int64)
    # bucket order: early-collective buckets (0,1) first, largest bucket last
    bucket_tiles = tiles.sum(axis=1)
    early = [b for b in range(NB) if (b + 1) * BUCKET <= SPLIT_W * C * WIN]
    rest = sorted((b for b in range(NB) if b not in early),
                  key=lambda b: (bucket_tiles[b], b))
    border = early + rest
    # group schedule shared across cores
    groups = []  # (bucket, window, n_tiles, tile_start)
    tpos = 0
    for b in border:
        for w in range(NW):
            t = int(tiles[b, w])
            if t == 0:
                continue
            groups.append((b, w, t, tpos))
            tpos += t
    TOT_T = tpos
    TOT = TOT_T * 128

    # last group index per window (dense fires after this group's flush);
    # first group index per window (self-term write issued when it opens)
    lastgroup = {}
    firstgroup = {}
    for gi, (b, w, t, ts) in enumerate(groups):
        lastgroup[w] = gi
        if w not in firstgroup:
            firstgroup[w] = gi

    # fill per-core streams
    idx16 = np.zeros((C, TOT), dtype=np.int16)
    dstloc = np.full((C, TOT), -1.0, dtype=np.float32)
    wvals = np.zeros((C, L, TOT), dtype=np.float32)
    for c in range(C):
        s_c, d_c, w_c, b_c, wl_c = per_core[c]
        # edges sorted by (b, w) lexicographic; groups are in border order
        starts = {}
        epos = 0
        for b in range(NB):
            for w in range(NW):
                n = int(counts[c, b, w])
                starts[(b, w)] = (epos, n)
                epos += n
        assert epos == len(s_c)
        for (b, w, t, tstart) in groups:
            epos, n = starts[(b, w)]
            if n:
                sl = slice(epos, epos + n)
                o = tstart * 128
                idx16[c, o:o + n] = (s_c[sl] - b * BUCKET).astype(np.int16)
                dstloc[c, o:o + n] = (d_c[sl] - (c * SLICE + w * WIN)).astype(np.float32)
                wvals[c, :, o:o + n] = w_c[:, sl]

    # chunks: consecutive tile runs within one bucket (in border order);
    # the final tiles of the last bucket are tapered into small chunks so the
    # kernel tail (SDMA drain of the last chunk + dense of the last windows)
    # stays short
    chunks = []  # (bucket, tile_start, n_tiles)
    for b in border:
        bt = [g for g in groups if g[0] == b]
        if not bt:
            continue
        b0 = bt[0][3]
        bn = bt[-1][3] + bt[-1][2]
        t = b0
        while t < bn:
            ct = min(CH, bn - t)
            if b == border[-1] and bn - t <= CH:
                ct = min(8, bn - t)
            chunks.append((b, t, ct))
            t += ct

    # mark chunk-trailing pad slots with idx -1: the Q7 desc-gen kernel trims
    # trailing negatives per call, skipping their descriptors on cores with
    # fewer real edges (interior pads stay 0 -- gathered then zeroed by w=0)
    pad = dstloc < 0  # [C, TOT]  (trim disabled for bisect)

    # device layouts
    # wrapped gather indices: edge i -> [i % 16, i // 16], replicated x8
    idx_wrapped = np.zeros((C, 128, TOT // 16), dtype=np.int16)
    for c in range(C):
        w16 = idx16[c].reshape(TOT // 16, 16).T  # [16, TOT//16]
        idx_wrapped[c] = np.tile(w16, (8, 1))
    # per-tile-major: [128, TOT_T]: (p, t) = edge t*128+p
    dstloc_t = np.transpose(dstloc.reshape(C, TOT_T, 128), (0, 2, 1)).astype(BF)
    wvals_t = np.transpose(wvals.reshape(C, L, TOT_T, 128), (0, 1, 3, 2)).astype(BF)

    # node table: bf16 padded to 128 features (256B rows), rows in gid order
    x_pad = np.zeros((NPAD, 128), dtype=BF)
    x_pad[gid_of[:N], :D] = np.asarray(x, dtype=np.float32).astype(BF)
    x_f32 = np.zeros((NPAD, D), dtype=np.float32)
    x_f32[:N] = np.asarray(x, dtype=np.float32)
    x_own = np.transpose(
        x_f32.reshape(C, NW, 128, D), (0, 2, 1, 3)).copy()  # [C, 128, NW, 64]

    iota = np.broadcast_to(np.arange(128, dtype=np.float32), (128, 1, 128)).astype(BF)
    iota_b = np.broadcast_to(np.arange(128, dtype=np.float32),
                             (128, CH, 128)).astype(BF)
    id64 = np.eye(64, dtype=np.float32)
    id128 = np.eye(128, dtype=np.float32)
    lwT = np.transpose(np.asarray(lin_w, dtype=np.float32), (0, 2, 1)).astype(BF).copy()
    fwT = np.asarray(fc_w, dtype=np.float32).T.astype(BF).copy()

    gamma = np.asarray(gamma, dtype=np.float32)
    beta = np.asarray(beta, dtype=np.float32)
    ln_trivial = bool(np.all(gamma == 1.0) and np.all(beta == 0.0))

    meta = dict(N=N, NW=NW, SLICE=SLICE, NPAD=NPAD, NB=NB, TOT_T=TOT_T,
                groups=tuple(groups), chunks=tuple(chunks),
                lastgroup=tuple(sorted(lastgroup.items())),
                firstgroup=tuple(sorted(firstgroup.items())),
                ln_trivial=ln_trivial)

    in_maps = []
    for c in range(C):
        in_maps.append({
            "x_pad": x_pad,
            "x_own": x_own[c],
            "idx_w": idx_wrapped[c],
            "dstloc": dstloc_t[c],
            "wv": wvals_t[c],
            "nwdeg": nwdeg[c],
            "iota": iota,
            "iota_b": iota_b,
            "id64": id64,
            "id128": id128,
            "lwT": lwT,
            "lb": np.asarray(lin_b, dtype=np.float32),
            "fwT": fwT,
            "fb": np.asarray(fc_b, dtype=np.float32).reshape(64, 1),
            "gm": np.broadcast_to(gamma[:, None, :], (L, 128, D)).copy(),
            "bt": np.broadcast_to(beta[:, None, :], (L, 128, D)).copy(),
        })
    return meta, in_maps


def _split_multi_waits(nc, mybir):
    """This walrus build rejects >1 sync-wait per instruction; hoist extras
    onto single-wait NOPs inserted just before, same engine."""
    ctr = 0
    for bbw in nc.bb_map.values():
        bb = bbw.bb
        insts = bb.instructions
        new = []
        changed = False
        for inst in insts:
            si = inst.sync_info
            waits = list(si.on_wait) if si and si.on_wait else []
            if len(waits) > 1:
                changed = True
                for w in waits[:-1]:
                    ctr += 1
                    new.append(mybir.InstNoOp(
                        name=f"I-waitsplit-{ctr}",
                        engine=inst.engine,
                        sync_info=mybir.SyncInfo(on_wait=[w], on_update=[]),
                    ))
                si.on_wait = [waits[-1]]
            new.append(inst)
        if changed:
            bb.instructions = new
    return nc


def _build(meta, split_waits=True, n_layers=L):
    import concourse.bass as bass
    import concourse.mybir as mybir
    from concourse import library_config
    from concourse.library_overlay import lower_extended_insts
    from concourse.tile import TileContext

    NW = meta["NW"]
    SLICE = meta["SLICE"]
    NPAD = meta["NPAD"]
    NB = meta["NB"]
    TOT_T = meta["TOT_T"]
    groups = meta["groups"]
    chunks = meta["chunks"]
    lastgroup = dict(meta["lastgroup"])
    firstgroup = dict(meta["firstgroup"])
    ln_trivial = meta["ln_trivial"]
    TOT = TOT_T * 128
    # block index after which the early AllGather (windows < SPLIT_W) fires
    ABLK = math.ceil(SPLIT_W / BLK) - 1

    F32 = mybir.dt.float32
    BF = mybir.dt.bfloat16
    I16 = mybir.dt.int16
    AF = mybir.ActivationFunctionType
    OP = mybir.AluOpType

    nc = bass.Bass(num_devices=C, num_swdge_queues=4)

    x_pad = nc.declare_dram_parameter("x_pad", [NPAD, 128], BF, isOutput=False)
    x_own = nc.declare_dram_parameter("x_own", [128, NW, D], F32, isOutput=False)
    idx_w = nc.declare_dram_parameter("idx_w", [128, TOT // 16], I16, isOutput=False)
    dstloc = nc.declare_dram_parameter("dstloc", [128, TOT_T], BF, isOutput=False)
    wv = nc.declare_dram_parameter("wv", [L, 128, TOT_T], BF, isOutput=False)
    nwdeg = nc.declare_dram_parameter("nwdeg", [128, L, NW], F32, isOutput=False)
    iota = nc.declare_dram_parameter("iota", [128, 1, 128], BF, isOutput=False)
    iota_b = nc.declare_dram_parameter("iota_b", [128, CH, 128], BF, isOutput=False)
    id64 = nc.declare_dram_parameter("id64", [64, 64], F32, isOutput=False)
    id128 = nc.declare_dram_parameter("id128", [128, 128], F32, isOutput=False)
    lwT = nc.declare_dram_parameter("lwT", [L, 64, 64], BF, isOutput=False)
    lb = nc.declare_dram_parameter("lb", [L, 64], F32, isOutput=False)
    fwT = nc.declare_dram_parameter("fwT", [64, 64], BF, isOutput=False)
    fb = nc.declare_dram_parameter("fb", [64, 1], F32, isOutput=False)
    if not ln_trivial:
        gm = nc.declare_dram_parameter("gm", [L, 128, 64], F32, isOutput=False)
        bt = nc.declare_dram_parameter("bt", [L, 128, 64], F32, isOutput=False)
    out = nc.declare_dram_parameter("out", [128, NW, D], F32, isOutput=True)

    LO_ROWS = SPLIT_W * C * WIN
    HI_ROWS = NPAD - LO_ROWS
    if n_layers > 1:
        tabs_lo = [
            nc.dram_tensor("tabA_lo", [LO_ROWS, 128], BF, addr_space="Shared"),
            nc.dram_tensor("tabB_lo", [LO_ROWS, 128], BF, addr_space="Shared"),
        ]
        tabs_hi = [
            nc.dram_tensor("tabA_hi", [HI_ROWS, 128], BF, addr_space="Shared"),
            nc.dram_tensor("tabB_hi", [HI_ROWS, 128], BF, addr_space="Shared"),
        ]
        slice_outs = [
            nc.dram_tensor("slice0", [SLICE, 128], BF),
            nc.dram_tensor("slice1", [SLICE, 128], BF),
        ]
    else:
        tabs_lo, tabs_hi, slice_outs = [x_pad, x_pad], [x_pad, x_pad], []

    nc.gpsimd.load_library(library_config.mlp)

    with TileContext(nc) as tc:
        with (
            tc.tile_pool(name="const", bufs=1) as cpool,
            tc.tile_pool(name="big", bufs=1) as bigp,
            tc.tile_pool(name="gat", bufs=6) as gpool,
            tc.tile_pool(name="msg", bufs=4) as mpool,
            tc.tile_pool(name="oh", bufs=3) as opool,
            tc.tile_pool(name="strm", bufs=8) as stp,
            tc.tile_pool(name="dense", bufs=3) as dpool,
            tc.tile_pool(name="lnp", bufs=1) as lnp,
            tc.tile_pool(name="stgp", bufs=2) as stgp,
            tc.tile_pool(name="psagg", bufs=3, space="PSUM") as ps_agg,
            tc.tile_pool(name="psd", bufs=2, space="PSUM") as ps_d,
            tc.tile_pool(name="pst", bufs=3, space="PSUM") as ps_t,
        ):
            # constants
            iota_t = cpool.tile([128, 1, 128], BF)
            nc.sync.dma_start(out=iota_t[:], in_=iota[:, :, :])
            iota_bt = cpool.tile([128, CH, 128], BF)
            nc.sync.dma_start(out=iota_bt[:], in_=iota_b[:, :, :])
            id64_t = cpool.tile([64, 64], F32)
            nc.sync.dma_start(out=id64_t[:], in_=id64[:, :])
            id128_t = cpool.tile([128, 128], F32)
            nc.sync.dma_start(out=id128_t[:], in_=id128[:, :])
            nwdeg_t = cpool.tile([128, L, NW], F32)
            nc.sync.dma_start(out=nwdeg_t[:], in_=nwdeg[:, :, :])
            lwT_ts = []
            for l in range(L):
                t = cpool.tile([64, 64], BF, tag=f"lwT{l}")
                nc.sync.dma_start(out=t[:], in_=lwT[l, :, :])
                lwT_ts.append(t)
            lb_ts = []
            for l in range(L):
                t = cpool.tile([64, 1], F32, tag=f"lb{l}")
                nc.sync.dma_start(out=t[:], in_=lb[l, :, None])
                lb_ts.append(t)
            fwT_t = cpool.tile([64, 64], BF)
            nc.sync.dma_start(out=fwT_t[:], in_=fwT[:, :])
            fb_t = cpool.tile([64, 1], F32)
            nc.sync.dma_start(out=fb_t[:], in_=fb[:, :])
            gm_ts, bt_ts = [], []
            if not ln_trivial:
                for l in range(L):
                    g_ = cpool.tile([128, 64], F32, tag=f"gm{l}")
                    nc.sync.dma_start(out=g_[:], in_=gm[l, :, :])
                    gm_ts.append(g_)
                    b_ = cpool.tile([128, 64], F32, tag=f"bt{l}")
                    nc.sync.dma_start(out=b_[:], in_=bt[l, :, :])
                    bt_ts.append(b_)

            eps_t = cpool.tile([128, 1], F32)
            nc.vector.memset(eps_t[:], EPS)
            # one register per distinct gather size, reused across all calls
            nidx_regs = {}
            for (_b, _t0, _ct) in chunks:
                v = _ct * 128
                if v not in nidx_regs:
                    nidx_regs[v] = nc.gpsimd.to_reg(v)

            own = [bigp.tile([128, NW, D], F32, tag="own_a", name="own_a"),
                   bigp.tile([128, NW, D], F32, tag="own_b", name="own_b")]
            nc.sync.dma_start(out=own[0][:], in_=x_own[:, :, :])
            agg = bigp.tile([64, NW * 128], BF, tag="agg", name="agg")

            for l in range(n_layers):
                own_cur = own[l % 2]
                own_nxt = own[(l + 1) % 2]
                last = l == n_layers - 1

                BLKL = BLK_LAST if l == n_layers - 1 else BLK
                NBLK = math.ceil(NW / BLKL)
                win_done = set()
                self_done = set()
                blocks_done = [0]

                def self_write(w):
                    # agg[:, w] = transpose(own_cur[:, w, :] * -wdeg)
                    # (first write of the window; flushes add on top)
                    sc = dpool.tile([128, 64], F32, tag="sc", name="sc")
                    nc.vector.tensor_tensor(
                        out=sc[:], in0=own_cur[:, w, :],
                        in1=nwdeg_t[:, l, w, None].to_broadcast([128, 64]),
                        op=OP.mult)
                    pt = ps_t.tile([64, 128], F32, tag="pst", name="pst")
                    nc.tensor.transpose(pt[:], sc[:], id128_t[:])
                    nc.scalar.copy(agg[:, w * 128:(w + 1) * 128], pt[:])
                    self_done.add(w)

                def dense_window(w):
                    pd = ps_d.tile([64, 128], F32, tag="psd", name="psd")
                    nc.tensor.matmul(pd[:], lhsT=lwT_ts[l][:],
                                     rhs=agg[:, w * 128:(w + 1) * 128],
                                     start=True, stop=True)
                    rT = dpool.tile([64, 128], F32, tag="rT", name="rT")
                    nc.scalar.activation(rT[:], pd[:], AF.Relu,
                                         bias=lb_ts[l][:, 0:1])
                    pt = ps_t.tile([128, 64], F32, tag="pst", name="pst")
                    nc.tensor.transpose(pt[:], rT[:], id64_t[:])
                    nc.scalar.copy(own_nxt[:, w, :], pt[:])

                def finish_block(k):
                    w0 = k * BLKL
                    w1 = min(w0 + BLKL, NW)
                    nb = w1 - w0
                    blk = own_nxt[:, w0:w1, :]
                    mu_s = dpool.tile([128, BLK], F32, tag="mu", name="mu")
                    nc.vector.tensor_reduce(mu_s[:, :nb], blk,
                                            axis=mybir.AxisListType.X, op=OP.add)
                    sq = lnp.tile([128, BLK, D], BF, tag="sq", name="sq")
                    nc.scalar.activation(sq[:, :nb, :], blk, AF.Square)
                    ssq = dpool.tile([128, BLK], F32, tag="ssq", name="ssq")
                    nc.vector.tensor_reduce(ssq[:, :nb], sq[:, :nb, :],
                                            axis=mybir.AxisListType.X, op=OP.add)
                    a2 = dpool.tile([128, BLK], F32, tag="a2", name="a2")
                    nc.vector.tensor_tensor(out=a2[:, :nb], in0=mu_s[:, :nb],
                                            in1=mu_s[:, :nb], op=OP.mult)
                    bvar = dpool.tile([128, BLK], F32, tag="bvar", name="bvar")
                    nc.vector.scalar_tensor_tensor(
                        out=bvar[:, :nb], in0=a2[:, :nb], scalar=-1.0 / D,
                        in1=ssq[:, :nb], op0=OP.mult, op1=OP.add)
                    std = dpool.tile([128, BLK], F32, tag="std", name="std")
                    nc.scalar.activation(std[:, :nb], bvar[:, :nb], AF.Sqrt,
                                         bias=eps_t[:, 0:1], scale=1.0 / D)
                    rstd = dpool.tile([128, BLK], F32, tag="rstd", name="rstd")
                    nc.vector.reciprocal(rstd[:, :nb], std[:, :nb])
                    xc = lnp.tile([128, BLK, D], BF, tag="xc", name="xc")
                    nc.vector.scalar_tensor_tensor(
                        out=xc[:, :nb, :],
                        in0=mu_s[:, :nb, None].to_broadcast([128, nb, D]),
                        scalar=-1.0 / D, in1=blk,
                        op0=OP.mult, op1=OP.add)
                    nc.vector.tensor_tensor(
                        out=blk, in0=xc[:, :nb, :],
                        in1=rstd[:, :nb, None].to_broadcast([128, nb, D]),
                        op=OP.mult)
                    if not ln_trivial:
                        nc.vector.tensor_tensor(
                            out=blk, in0=blk,
                            in1=gm_ts[l][:, None, :].to_broadcast([128, nb, D]),
                            op=OP.mult)
                        nc.vector.tensor_tensor(
                            out=blk, in0=blk,
                            in1=bt_ts[l][:, None, :].to_broadcast([128, nb, D]),
                            op=OP.add)
                    nc.vector.tensor_tensor(out=blk, in0=blk,
                                            in1=own_cur[:, w0:w1, :], op=OP.add)
                    if not last:
                        # write padded bf16 slice rows for the next table
                        stg = stgp.tile([128, BLK, 128], BF, tag="stg", name="stg")
                        nc.scalar.copy(stg[:, :nb, 0:64], blk)
                        so_ap = slice_outs[l].ap().rearrange(
                            "(w p) f -> p w f", p=128)
                        nc.sync.dma_start(out=so_ap[:, w0:w1, :],
                                          in_=stg[:, :nb, :])
                        # early AllGather: windows [0, SPLIT_W) feed buckets
                        # 0..1 of the next layer; fires mid-gather-phase and
                        # overlaps the remaining gathers of this layer
                        if k == ABLK:
                            nc.gpsimd.collective_compute(
                                "AllGather",
                                mybir.AluOpType.bypass,
                                replica_groups=[list(range(C))],
                                ins=[slice_outs[l][0:SPLIT_W * 128, :].opt()],
                                outs=[tabs_lo[l][:].opt()],
                            )
                    else:
                        # final fc on this block, then store to out
                        for w in range(w0, w1):
                            ptf = ps_t.tile([64, 128], F32, tag="pst", name="pst")
                            nc.tensor.transpose(ptf[:], own_nxt[:, w, :],
                                                id128_t[:])
                            hT = dpool.tile([64, 128], BF, tag="hT", name="hT")
                            nc.scalar.copy(hT[:], ptf[:])
                            po = ps_d.tile([64, 128], F32, tag="psd", name="psd")
                            nc.tensor.matmul(po[:], lhsT=fwT_t[:], rhs=hT[:],
                                             start=True, stop=True)
                            ob = dpool.tile([64, 128], F32, tag="ob", name="ob")
                            nc.vector.tensor_scalar_add(ob[:], po[:],
                                                        fb_t[:, 0:1])
                            pq = ps_t.tile([128, 64], F32, tag="pst", name="pst")
                            nc.tensor.transpose(pq[:], ob[:], id64_t[:])
                            nc.scalar.copy(own_cur[:, w, :], pq[:])
                        nc.sync.dma_start(out=out[:, w0:w1, :],
                                          in_=own_cur[:, w0:w1, :])

                open_ps = {}
                for ci_, (b, t0, ct) in enumerate(chunks):
                    nidx = ct * 128
                    idx_t = stp.tile([128, CH * 8], I16, tag="idx", name="idx")
                    nc.sync.dma_start(out=idx_t[:, :ct * 8],
                                      in_=idx_w[:, t0 * 8:(t0 + ct) * 8])
                    dst_t = stp.tile([128, CH], BF, tag="dst", name="dst")
                    nc.scalar.dma_start(out=dst_t[:, :ct],
                                        in_=dstloc[:, t0:t0 + ct])
                    w_t = stp.tile([128, CH], BF, tag="w", name="w")
                    nc.scalar.dma_start(out=w_t[:, :ct],
                                        in_=wv[l, :, t0:t0 + ct])

                    gat = gpool.tile([128, CH, 128], BF, tag="gat", name="gat")
                    brows = min(BUCKET, NPAD - b * BUCKET)
                    if l == 0:
                        tab_ap = x_pad[b * BUCKET:b * BUCKET + brows, :]
                    elif b * BUCKET < LO_ROWS:
                        tab_ap = tabs_lo[l - 1][b * BUCKET:b * BUCKET + brows, :]
                    else:
                        r0 = b * BUCKET - LO_ROWS
                        tab_ap = tabs_hi[l - 1][r0:r0 + brows, :]
                    nc.gpsimd.dma_gather(
                        out_ap=gat[:, :ct, :],
                        in_ap=tab_ap,
                        idxs_ap=idx_t[:, :ct * 8],
                        num_idxs=nidx,
                        num_idxs_reg=nidx_regs[nidx],
                        elem_size=128,
                        single_packet=False,
                        queue_num=ci_ % 4,
                    )
                    # scale messages (features 0:64 of each gathered row)
                    msgs = mpool.tile([128, CH, 64], BF, tag="msgs", name="msgs")
                    nc.vector.tensor_tensor(
                        out=msgs[:, :ct, :],
                        in0=gat[:, :ct, 0:64],
                        in1=w_t[:, :ct, None].to_broadcast([128, ct, 64]),
                        op=OP.mult,
                    )
                    oh = opool.tile([128, CH, 128], BF, tag="oh", name="oh")
                    nc.vector.tensor_tensor(
                        out=oh[:, :ct, :],
                        in0=dst_t[:, :ct, None].to_broadcast([128, ct, 128]),
                        in1=iota_bt[:, :ct, :],
                        op=OP.is_equal,
                    )
                    # matmuls per tile
                    for gi, (gb, gw, gt, gstart) in enumerate(groups):
                        if gstart + gt <= t0 or gstart >= t0 + ct:
                            continue
                        lo = max(gstart, t0)
                        hi = min(gstart + gt, t0 + ct)
                        if gstart >= t0:
                            open_ps[gi] = ps_agg.tile([64, 128], F32,
                                                      tag="psagg", name="psagg")
                            if firstgroup.get(gw) == gi:
                                self_write(gw)
                        ps = open_ps[gi]
                        for t in range(lo, hi):
                            ti = t - t0
                            nc.tensor.matmul(
                                ps[:],
                                lhsT=msgs[:, ti, :],
                                rhs=oh[:, ti, :],
                                start=(t == gstart),
                                stop=(t == gstart + gt - 1),
                            )
                        if gstart + gt <= t0 + ct:
                            # group complete: flush into agg
                            nc.vector.tensor_tensor(
                                out=agg[:, gw * 128:(gw + 1) * 128],
                                in0=agg[:, gw * 128:(gw + 1) * 128],
                                in1=ps[:],
                                op=OP.add,
                            )
                            del open_ps[gi]
                            # window complete -> dense; block complete -> LN
                            if lastgroup.get(gw) == gi:
                                dense_window(gw)
                                win_done.add(gw)
                                while (blocks_done[0] < NBLK and all(
                                        w_ in win_done for w_ in
                                        range(blocks_done[0] * BLKL,
                                              min((blocks_done[0] + 1) * BLKL,
                                                  NW)))):
                                    finish_block(blocks_done[0])
                                    blocks_done[0] += 1
                assert not open_ps

                # windows with no groups at all (shouldn't happen, but safe)
                for w in range(NW):
                    if w not in win_done:
                        if w not in self_done:
                            self_write(w)
                        dense_window(w)
                        win_done.add(w)
                while blocks_done[0] < NBLK:
                    finish_block(blocks_done[0])
                    blocks_done[0] += 1

                if not last:
                    nc.gpsimd.collective_compute(
                        "AllGather",
                        mybir.AluOpType.bypass,
                        replica_groups=[list(range(C))],
                        ins=[slice_outs[l][SPLIT_W * 128:SLICE, :].opt()],
                        outs=[tabs_hi[l][:].opt()],
                    )

    if split_waits:
        _split_multi_waits(nc, mybir)
    lower_extended_insts(nc)
    return nc


def kernel(**inputs):
    from concourse.bass_utils import run_bass_kernel_spmd

    x = np.asarray(inputs["x"])
    meta, in_maps = _prep(
        x, np.asarray(inputs["edge_index"]), np.asarray(inputs["edge_attr"]),
        np.asarray(inputs["lin_w"]), np.asarray(inputs["lin_b"]),
        np.asarray(inputs["emlp_w"]), np.asarray(inputs["emlp_b"]),
        np.asarray(inputs["gamma"]), np.asarray(inputs["beta"]),
        np.asarray(inputs["fc_w"]), np.asarray(inputs["fc_b"]))

    key = (meta["NW"], meta["TOT_T"], meta["groups"], meta["chunks"],
           meta["ln_trivial"])
    if key not in _CACHE:
        _CACHE[key] = _build(meta)
    nc = _CACHE[key]

    res = run_bass_kernel_spmd(nc, in_maps, list(range(C)))
    N = meta["N"]
    NW = meta["NW"]
    parts = []
    for c in range(C):
        o = np.asarray(res.results[c]["out"])  # [128, NW, 64]
        parts.append(np.transpose(o, (1, 0, 2)).reshape(NW * 128, D))
    full = np.concatenate(parts, axis=0)[:N]
    return full.astype(np.float32)
